# revision 17
# baseline (speedup 1.0000x reference)
"""Two-layer GAT (PyG-style GATConv x2) on 8 Trainium2 NeuronCores — v2.

Sharding: nodes (and incident edges, by destination) sharded across 8
cores; small weights replicated. Per-edge source rows fetched via SWDGE
dma_gather from a row-major node table in HBM (bf16 h rows for layer 0,
fp32 h1 rows for layer 1 — both exactly 512 B / 256 B per row, the
gather's minimum-efficient granularity). Edges are dst-sorted and
grouped per 128-row dst tile; each 128-edge chunk is segment-reduced
with a one-hot matmul into PSUM.

v2 changes vs v1 (2.14 ms):
 - Node tables carry ONLY features. Attention alphas are emitted as
   separate per-node fp32 tables; the host expands them per edge (pure
   fancy-indexing, same category as v1's alpha_dst expansion) so logits
   are assembled on-chip from two sequentially-streamed fp32 inputs.
   Layer-0 gather rows shrink 768 B -> 512 B.
 - Softmax denominators accumulate via a second tiny matmul per chunk
   (lhsT = exp-weights) instead of embedding weight columns in the
   gathered payload — kills v1's pathological strided DVE copy
   (~380 us/launch).
 - All PSUM->SBUF evacuation copies moved to the idle Scalar engine.
 - Gather calls: 16 chunks per call, striped round-robin across all 4
   SWDGE queues.
"""

import os

import numpy as np

import concourse.bacc as bacc
import concourse.mybir as mybir
from concourse import tile
from concourse.bass_utils import run_bass_kernel_spmd

fp32 = mybir.dt.float32
bf16 = mybir.dt.bfloat16
i16 = mybir.dt.int16
Alu = mybir.AluOpType
Act = mybir.ActivationFunctionType

NCORES = 8
NEG_SLOPE = 0.2
EPS = 1e-16
CPC = 8  # 128-edge chunks per gather call (1024 idx — SWDGE ring limit)


def _dims_full():
    return dict(
        N=50000,
        NLOC=6250,
        NLOC_PAD=6272,
        F_IN=256,
        HID=256,
        H=4,
        DH=64,
        C_OUT=64,
        ELEM0=256,  # bf16 h row -> 512 B
        ELEM1=64,  # fp32 h1 row -> 256 B
        SPLIT=32768,  # int16 gather-index split point
    )


# ---------------------------------------------------------------- launch 1


def build_phase_a(d):
    """Per core: h0 = x_shard @ W0 -> bf16 table0 rows; alphas -> fp32 atab0."""
    nc = bacc.Bacc(None, target_bir_lowering=False, debug=False, num_swdge_queues=4)
    NP, F, HID = d["NLOC_PAD"], d["F_IN"], d["HID"]
    assert F == 256 and HID == 256

    xT = nc.dram_tensor("xT", [F, NP], fp32, kind="ExternalInput")
    W0 = nc.dram_tensor("W0", [F, HID], fp32, kind="ExternalInput")
    A0 = nc.dram_tensor("A0", [HID, 8], fp32, kind="ExternalInput")
    eye = nc.dram_tensor("eye", [128, 128], fp32, kind="ExternalInput")
    table0 = nc.dram_tensor("table0", [NP, 256], bf16, kind="ExternalOutput")
    atab0 = nc.dram_tensor("atab0", [NP, 8], fp32, kind="ExternalOutput")

    TW = 512
    n_t = (NP + TW - 1) // TW

    with tile.TileContext(nc) as tc:
        with (
            tc.tile_pool(name="const", bufs=1) as cpool,
            tc.tile_pool(name="work", bufs=3) as pool,
            tc.tile_pool(name="psum", bufs=1, space="PSUM") as pp,
            tc.tile_pool(name="psum1", bufs=2, space="PSUM") as pp1,
        ):
            w0_sb = [
                cpool.tile([128, HID], fp32, tag=f"w0_{k}", name=f"w0_{k}")
                for k in range(2)
            ]
            a0_sb = [
                cpool.tile([128, 8], fp32, tag=f"a0_{k}", name=f"a0_{k}")
                for k in range(2)
            ]
            eye_sb = cpool.tile([128, 128], fp32)
            for k in range(2):
                nc.sync.dma_start(w0_sb[k][:], W0[128 * k : 128 * (k + 1), :])
                nc.sync.dma_start(a0_sb[k][:], A0[128 * k : 128 * (k + 1), :])
            nc.sync.dma_start(eye_sb[:], eye[:])

            for t in range(n_t):
                c0 = t * TW
                cw = min(TW, NP - c0)
                xt = [
                    pool.tile([128, TW], fp32, tag=f"xt{k}", name=f"xt{k}")
                    for k in range(2)
                ]
                for k in range(2):
                    nc.sync.dma_start(
                        xt[k][:, :cw], xT[128 * k : 128 * (k + 1), c0 : c0 + cw]
                    )
                hT = [
                    pool.tile([128, TW], fp32, tag=f"ht{m}", name=f"ht{m}")
                    for m in range(2)
                ]
                for m in range(2):
                    ps = pp.tile([128, TW], fp32, tag=f"ps{m}", name=f"ps{m}")
                    for k in range(2):
                        nc.tensor.matmul(
                            ps[:, :cw],
                            w0_sb[k][:, 128 * m : 128 * (m + 1)],
                            xt[k][:, :cw],
                            start=(k == 0),
                            stop=(k == 1),
                        )
                    nc.scalar.activation(hT[m][:, :cw], ps[:, :cw], Act.Copy)

                nq = (cw + 127) // 128
                for q in range(nq):
                    q0 = q * 128
                    qw = min(128, cw - q0)
                    pa = pp1.tile([128, 8], fp32, tag="pa")
                    for k in range(2):
                        nc.tensor.matmul(
                            pa[:qw, :],
                            hT[k][:, q0 : q0 + qw],
                            a0_sb[k][:],
                            start=(k == 0),
                            stop=(k == 1),
                        )
                    R = pool.tile([128, 256], bf16, tag="rows")
                    for m in range(2):
                        pt = pp1.tile([128, 128], fp32, tag=f"pt{m}", name=f"pt{m}")
                        nc.tensor.transpose(
                            pt[:qw, :], hT[m][:, q0 : q0 + qw], eye_sb[:]
                        )
                        nc.scalar.activation(
                            R[:qw, 128 * m : 128 * (m + 1)], pt[:qw, :], Act.Copy
                        )
                    paS = pool.tile([128, 8], fp32, tag="paS")
                    nc.scalar.activation(paS[:qw, :], pa[:qw, :], Act.Copy)
                    r0 = c0 + q0
                    nc.sync.dma_start(table0[r0 : r0 + qw, :], R[:qw, :])
                    nc.sync.dma_start(atab0[r0 : r0 + qw, :], paS[:qw, :])
    nc.compile()
    return nc


# ------------------------------------------------------------ edge machinery


def _edge_pass(nc, tc, d, table, streams_dram, elem, gdt, nfeat, nhead, fin, pp):
    """Dst-sorted edge pass. Per gather call (CPC chunks of 128 edges):
    fetch source rows (SWDGE gather, round-robin over the 4 queues),
    assemble logits from the two host-expanded per-edge alpha streams,
    leaky-relu + exp, build one-hot via is_equal, weight the payload.
    Per chunk: main one-hot matmul accumulates the weighted messages per
    dst tile; a second tiny matmul (lhsT = exp-weights) accumulates the
    softmax denominators as dnT [nhead, 128]."""
    NP, SPLIT, NROWS = d["NLOC_PAD"], d["SPLIT"], d["N_TAB"]
    K_LO, K_HI = d["K_LO"], d["K_HI"]
    NT = NP // 128
    qcnt = [0]

    with (
        tc.tile_pool(name="eidx", bufs=1) as ipool,
        tc.tile_pool(name="edge", bufs=3) as pool,
    ):
        iota_sb = ipool.tile([128, CPC, 128], bf16)
        nc.sync.dma_start(iota_sb[:], d["iota_dram"][:])
        streams = []
        for s, K in ((0, K_LO), (1, K_HI)):
            gi_d, rr_d, as_d, ad_d = streams_dram[s]
            nch = NT * K
            gi = ipool.tile([128, nch * 8], i16, name=f"gi{s}")
            rr = ipool.tile([128, nch], bf16, name=f"rr{s}")
            asx = ipool.tile([128, nch, nhead], fp32, name=f"as{s}")
            adx = ipool.tile([128, nch, nhead], fp32, name=f"ad{s}")
            nc.sync.dma_start(gi[:], gi_d[:])
            nc.sync.dma_start(rr[:], rr_d[:])
            nc.sync.dma_start(asx[:], as_d[:])
            nc.sync.dma_start(adx[:], ad_d[:])
            base = table[0:SPLIT, :] if s == 0 else table[SPLIT:NROWS, :]
            streams.append(
                dict(gi=gi, rr=rr, asx=asx, adx=adx, K=K, base=base, ncalls=0,
                     tiles={})
            )

        def emit_call(st, call):
            c0 = call * CPC
            nch = min(CPC, NT * st["K"] - c0)
            ne = nch * 128
            G = pool.tile([128, CPC, elem], gdt, tag="G", name="G", bufs=4)
            OH = pool.tile([128, CPC, 128], bf16, tag="OH", name="OH", bufs=4)
            nc.gpsimd.dma_gather(
                G[:, :nch, :],
                st["base"],
                st["gi"][:, c0 * 8 : c0 * 8 + ne // 16],
                ne,
                ne,
                elem,
                queue_num=qcnt[0] % 4,
            )
            qcnt[0] += 1
            ew = pool.tile([128, CPC, nhead], fp32, tag="ew", name="ew", bufs=4)
            nc.vector.tensor_tensor(
                ew[:, :nch, :],
                st["asx"][:, c0 : c0 + nch, :],
                st["adx"][:, c0 : c0 + nch, :],
                op=Alu.add,
            )
            nc.vector.scalar_tensor_tensor(
                ew[:, :nch, :],
                ew[:, :nch, :],
                NEG_SLOPE,
                ew[:, :nch, :],
                op0=Alu.mult,
                op1=Alu.max,
            )
            ewb = pool.tile([128, CPC, nhead], bf16, tag="ewb", name="ewb", bufs=4)
            nc.scalar.activation(ewb[:, :nch, :], ew[:, :nch, :], Act.Exp)
            # Pair-expanded rr: innermost stride-1 pairs keep DVE in its
            # 2x/4x packed modes (a stride-0 innermost operand forces 1x).
            rr2 = pool.tile([128, CPC, 2], bf16, tag="rr2", name="rr2", bufs=4)
            nc.vector.tensor_copy(
                rr2[:, :nch, :],
                st["rr"][:, c0 : c0 + nch].unsqueeze(2).broadcast_to(
                    [128, nch, 2]
                ),
            )
            oh2 = OH[:, :nch, :].rearrange("p c (q e) -> p c q e", q=64)
            io2 = iota_sb[:, :nch, :].rearrange("p c (q e) -> p c q e", q=64)
            rb2 = rr2[:, :nch, :].unsqueeze(2).broadcast_to([128, nch, 64, 2])
            nc.vector.tensor_tensor(oh2, rb2, io2, op=Alu.is_equal)
            if gdt == bf16:
                ew2 = pool.tile(
                    [128, CPC, nhead, 2], bf16, tag="ew2", name="ew2", bufs=4
                )
                nc.vector.tensor_copy(
                    ew2[:, :nch, :, :],
                    ewb[:, :nch, :].unsqueeze(3).broadcast_to(
                        [128, nch, nhead, 2]
                    ),
                )
                dh2 = nfeat // nhead // 2
                gm2 = G[:, :nch, :].rearrange(
                    "p c (h q e) -> p (c h) q e", h=nhead, q=dh2
                )
                wb2 = (
                    ew2[:, :nch, :, :]
                    .rearrange("p c h e -> p (c h) e")
                    .unsqueeze(2)
                    .broadcast_to([128, nch * nhead, dh2, 2])
                )
                nc.vector.tensor_tensor(gm2, gm2, wb2, op=Alu.mult)
                Gw = G
            else:
                Gw = pool.tile([128, CPC, elem], bf16, tag="Gw", name="Gw", bufs=4)
                wb = ewb[:, :nch, :].broadcast_to([128, nch, nfeat])
                nc.vector.tensor_tensor(Gw[:, :nch, :], G[:, :nch, :], wb,
                                        op=Alu.mult)
            return Gw, OH, ewb

        for t in range(NT):
            ps = pp.tile([128, nfeat], fp32, tag="ps", name="ps")
            dn = pp.tile([128, nhead], fp32, tag="dn", name="dn")
            first = True
            for st in streams:
                K = st["K"]
                for k in range(K):
                    c = t * K + k
                    call, cin = c // CPC, c % CPC
                    if call >= st["ncalls"]:
                        st["tiles"][call] = emit_call(st, call)
                        st["ncalls"] = call + 1
                        st["tiles"].pop(call - 3, None)
                    Gw, OH, ewb = st["tiles"][call]
                    last = st is streams[1] and k == K - 1
                    nc.tensor.matmul(
                        ps[:],
                        OH[:, cin, :],
                        Gw[:, cin, 0:nfeat],
                        start=first,
                        stop=last,
                        skip_group_check=True,
                    )
                    nc.tensor.matmul(
                        dn[:],
                        OH[:, cin, :],
                        ewb[:, cin, :],
                        start=first,
                        stop=last,
                        skip_group_check=True,
                    )
                    first = False
            fin(t, ps, dn)


# ---------------------------------------------------------------- launch 2


def build_layer0_edges(d):
    """Layer-0 edge pass with fused finalize (softmax-div + bias + ELU),
    then h1 = h0' @ W1 -> fp32 table1 rows + fp32 atab1."""
    nc = bacc.Bacc(None, target_bir_lowering=False, debug=False, num_swdge_queues=4)
    NP = d["NLOC_PAD"]
    HID, C_OUT, H, DH = d["HID"], d["C_OUT"], d["H"], d["DH"]
    NT = NP // 128

    table0 = nc.dram_tensor("table0", [d["N_TAB"], 256], bf16, kind="ExternalInput")
    sd = []
    for s, K in ((0, d["K_LO"]), (1, d["K_HI"])):
        nch = NT * K
        sd.append(
            (
                nc.dram_tensor(f"g{s}", [128, nch * 8], i16, kind="ExternalInput"),
                nc.dram_tensor(f"r{s}", [128, nch], bf16, kind="ExternalInput"),
                nc.dram_tensor(f"as{s}", [128, nch, H], fp32, kind="ExternalInput"),
                nc.dram_tensor(f"ad{s}", [128, nch, H], fp32, kind="ExternalInput"),
            )
        )
    iota = nc.dram_tensor("iota", [128, CPC, 128], bf16, kind="ExternalInput")
    W1b = nc.dram_tensor("W1b", [HID, C_OUT], bf16, kind="ExternalInput")
    A1e = nc.dram_tensor("A1e", [HID, 2], bf16, kind="ExternalInput")
    b0r = nc.dram_tensor("b0r", [128, HID], fp32, kind="ExternalInput")
    eye = nc.dram_tensor("eye", [128, 128], fp32, kind="ExternalInput")
    table1 = nc.dram_tensor("table1", [NP, 64], fp32, kind="ExternalOutput")
    atab1 = nc.dram_tensor("atab1", [NP, 2], fp32, kind="ExternalOutput")
    d = dict(d, iota_dram=iota)

    with tile.TileContext(nc) as tc:
        with (
            tc.tile_pool(name="fconst", bufs=1) as cpool,
            tc.tile_pool(name="fin", bufs=3) as pool,
            tc.tile_pool(name="h0all", bufs=1) as hpool,
            tc.tile_pool(name="epsum", bufs=2, space="PSUM") as pp,
        ):
            b0_sb = cpool.tile([128, HID], fp32)
            nc.sync.dma_start(b0_sb[:], b0r[:])
            eye_sb = cpool.tile([128, 128], fp32)
            nc.sync.dma_start(eye_sb[:], eye[:])
            H0 = hpool.tile([128, NT, HID], fp32)

            def fin0(t, ps, dn):
                dnS = pool.tile([128, H], fp32, tag="dnS", name="dnS")
                nc.vector.tensor_scalar_add(dnS[:], dn[:], EPS)
                recB = pool.tile([128, H], fp32, tag="recB", name="recB")
                nc.vector.reciprocal(recB[:], dnS[:])
                f4 = ps[:, 0:HID].rearrange("p (h e) -> p h e", h=H)
                rb = recB[:].unsqueeze(2).broadcast_to([128, H, DH])
                hrow = H0[:, t, :]
                nc.vector.tensor_tensor(
                    hrow.rearrange("p (h e) -> p h e", h=H), f4, rb, op=Alu.mult
                )
                nc.vector.tensor_tensor(hrow, hrow, b0_sb[:], op=Alu.add)
                tn = pool.tile([128, HID], fp32, tag="tn", name="tn")
                nc.vector.tensor_scalar_min(tn[:], hrow, 0.0)
                nc.scalar.activation(tn[:], tn[:], Act.Exp)
                tp = pool.tile([128, HID], fp32, tag="tp", name="tp")
                nc.vector.tensor_scalar_max(tp[:], hrow, 0.0)
                nc.vector.scalar_tensor_tensor(
                    hrow, tn[:], -1.0, tp[:], op0=Alu.add, op1=Alu.add
                )

            _edge_pass(nc, tc, d, table0, sd, 256, bf16, HID, H, fin0, pp)

            with (
                tc.tile_pool(name="tb1", bufs=3) as tpool,
                tc.tile_pool(name="tb1psum", bufs=1, space="PSUM") as pp2,
            ):
                w1_sb = [
                    cpool.tile([128, C_OUT], bf16, tag=f"w1_{k}", name=f"w1_{k}")
                    for k in range(2)
                ]
                a1e_sb = [
                    cpool.tile([128, 2], bf16, tag=f"a1e_{k}", name=f"a1e_{k}")
                    for k in range(2)
                ]
                for k in range(2):
                    nc.sync.dma_start(w1_sb[k][:], W1b[128 * k : 128 * (k + 1), :])
                    nc.sync.dma_start(a1e_sb[k][:], A1e[128 * k : 128 * (k + 1), :])

                for r in range(NT):
                    h0T = [
                        tpool.tile([128, 128], bf16, tag=f"h0T{k}", name=f"h0T{k}")
                        for k in range(2)
                    ]
                    for k in range(2):
                        pt = pp2.tile([128, 128], fp32, tag="pt", name="pt")
                        nc.tensor.transpose(
                            pt[:], H0[:, r, 128 * k : 128 * (k + 1)], eye_sb[:]
                        )
                        nc.scalar.activation(h0T[k][:], pt[:], Act.Copy)
                    pr1 = pp2.tile([128, C_OUT], fp32, tag="pr1", name="pr1")
                    pr2 = pp2.tile([128, 2], fp32, tag="pr2", name="pr2")
                    for k in range(2):
                        nc.tensor.matmul(
                            pr1[:],
                            h0T[k][:],
                            w1_sb[k][:],
                            start=(k == 0),
                            stop=(k == 1),
                            skip_group_check=True,
                        )
                        nc.tensor.matmul(
                            pr2[:],
                            h0T[k][:],
                            a1e_sb[k][:],
                            start=(k == 0),
                            stop=(k == 1),
                            skip_group_check=True,
                        )
                    R1 = tpool.tile([128, C_OUT], fp32, tag="R1", name="R1")
                    nc.scalar.activation(R1[:], pr1[:], Act.Copy)
                    palS = tpool.tile([128, 2], fp32, tag="palS", name="palS")
                    nc.scalar.activation(palS[:], pr2[:], Act.Copy)
                    nc.sync.dma_start(table1[128 * r : 128 * (r + 1), :], R1[:])
                    nc.sync.dma_start(atab1[128 * r : 128 * (r + 1), :], palS[:])
    nc.compile()
    return nc


# ---------------------------------------------------------------- launch 3


def build_layer1_edges(d):
    """Layer-1 edge pass with fused finalize -> output shard."""
    nc = bacc.Bacc(None, target_bir_lowering=False, debug=False, num_swdge_queues=4)
    NP, C_OUT = d["NLOC_PAD"], d["C_OUT"]
    NT = NP // 128

    table1 = nc.dram_tensor("table1", [d["N_TAB"], 64], fp32, kind="ExternalInput")
    sd = []
    for s, K in ((0, d["K_LO"]), (1, d["K_HI"])):
        nch = NT * K
        sd.append(
            (
                nc.dram_tensor(f"g{s}", [128, nch * 8], i16, kind="ExternalInput"),
                nc.dram_tensor(f"r{s}", [128, nch], bf16, kind="ExternalInput"),
                nc.dram_tensor(f"as{s}", [128, nch, 1], fp32, kind="ExternalInput"),
                nc.dram_tensor(f"ad{s}", [128, nch, 1], fp32, kind="ExternalInput"),
            )
        )
    iota = nc.dram_tensor("iota", [128, CPC, 128], bf16, kind="ExternalInput")
    eye = nc.dram_tensor("eye", [128, 128], fp32, kind="ExternalInput")
    b1r = nc.dram_tensor("b1r", [128, C_OUT], fp32, kind="ExternalInput")
    out = nc.dram_tensor("out", [NP, C_OUT], fp32, kind="ExternalOutput")
    d = dict(d, iota_dram=iota)

    with tile.TileContext(nc) as tc:
        with (
            tc.tile_pool(name="oconst", bufs=1) as cpool,
            tc.tile_pool(name="ofin", bufs=3) as pool,
            tc.tile_pool(name="epsum", bufs=2, space="PSUM") as pp,
        ):
            b1_sb = cpool.tile([128, C_OUT], fp32)
            nc.sync.dma_start(b1_sb[:], b1r[:])

            def fin1(t, ps, dn):
                dnS = pool.tile([128, 1], fp32, tag="dnS", name="dnS")
                nc.vector.tensor_scalar_add(dnS[:], dn[:], EPS)
                recB = pool.tile([128, 1], fp32, tag="recB", name="recB")
                nc.vector.reciprocal(recB[:], dnS[:])
                O = pool.tile([128, C_OUT], fp32, tag="O", name="O")
                rb = recB[:].broadcast_to([128, C_OUT])
                nc.vector.tensor_tensor(O[:], ps[:, 0:C_OUT], rb, op=Alu.mult)
                nc.vector.tensor_tensor(O[:], O[:], b1_sb[:], op=Alu.add)
                nc.sync.dma_start(out[128 * t : 128 * (t + 1), :], O[:])

            _edge_pass(nc, tc, d, table1, sd, 64, fp32, C_OUT, 1, fin1, pp)
    nc.compile()
    return nc


# ------------------------------------------------------------ host plumbing


def _wrap_idx(idx):
    """idx[j] -> [j%16, j//16], replicated across the 8 q7 core groups."""
    a = idx.reshape(-1, 16).T.astype(np.int16)
    return np.tile(a, (8, 1))


def _prep_edges(edge_index, d):
    """Partition edges by dst shard; per core split by src < SPLIT (int16
    gather range), group by 128-row dst tile (sorted by dst), pad each
    (tile, stream) segment to the global max chunk count K_LO / K_HI.

    Returns per core, per stream: (wrapped_idx, rrT, src_global, dst_global)
    where src/dst_global are [NT*K, 128] int64 node ids (0 on pad slots)."""
    N, NLOC, NP = d["N"], d["NLOC"], d["NLOC_PAD"]
    SPLIT = d["SPLIT"]
    NT = NP // 128
    src = np.concatenate([edge_index[0], np.arange(N, dtype=np.int64)])
    dst = np.concatenate([edge_index[1], np.arange(N, dtype=np.int64)])
    core = dst // NLOC
    per_core = []
    kmax = [1, 1]
    for c in range(NCORES):
        m = core == c
        s, t = src[m], dst[m] - c * NLOC
        order = np.argsort(t, kind="stable")
        s, t = s[order], t[order]
        lo = s < SPLIT
        segs = []
        for sm, base in ((lo, 0), (~lo, SPLIT)):
            ss, tt = s[sm], t[sm]
            counts = np.bincount(tt // 128, minlength=NT)
            segs.append((ss, tt, counts, base))
        per_core.append(segs)
        for si in range(2):
            kmax[si] = max(kmax[si], int(np.ceil(per_core[c][si][2].max() / 128)))
    K_LO, K_HI = kmax
    res = []
    for c in range(NCORES):
        arrs = []
        for si, K in ((0, K_LO), (1, K_HI)):
            ss, tt, counts, base = per_core[c][si]
            g = np.zeros((NT, K * 128), np.int64)
            sg = np.zeros((NT, K * 128), np.int64)
            dg = np.zeros((NT, K * 128), np.int64)
            rr = np.full((NT, K * 128), -1.0, np.float32)
            offs = np.concatenate([[0], np.cumsum(counts)])
            for tl in range(NT):
                n = counts[tl]
                g[tl, :n] = ss[offs[tl] : offs[tl] + n] - base
                sg[tl, :n] = ss[offs[tl] : offs[tl] + n]
                dg[tl, :n] = tt[offs[tl] : offs[tl] + n] + c * NLOC
                rr[tl, :n] = (tt[offs[tl] : offs[tl] + n] - 128 * tl).astype(
                    np.float32
                )
            arrs.append(
                (
                    _wrap_idx(g.ravel()),
                    np.ascontiguousarray(rr.reshape(NT * K, 128).T),
                    sg.reshape(NT * K, 128),
                    dg.reshape(NT * K, 128),
                )
            )
        res.append(arrs)
    return K_LO, K_HI, res


def _build_A0(att_src, att_dst):
    H, DH = att_src.shape
    A = np.zeros((H * DH, 2 * H), np.float32)
    for h in range(H):
        A[h * DH : (h + 1) * DH, h] = att_src[h]
        A[h * DH : (h + 1) * DH, H + h] = att_dst[h]
    return A


def _bf16(a):
    import ml_dtypes

    return a.astype(ml_dtypes.bfloat16)


_cache = {}
LAST_PROFILE = {}


def _run(nc, in_maps, core_ids, label):
    trace = bool(int(os.environ.get("GAT_PROFILE", "0")))
    if trace:
        try:
            import sys

            import profile_hook

            profile_hook.install()
            import concourse.bass_utils as bu

            bu.upload_artifacts = lambda tmpdir: "local://skipped"
            br = run_bass_kernel_spmd(nc, in_maps, core_ids, trace=True)
            LAST_PROFILE[label] = br.exec_time_ns
            return br.results
        except Exception as e:  # fall back to untraced
            print(f"traced run failed ({e!r}); untraced retry", file=sys.stderr)
    br = run_bass_kernel_spmd(nc, in_maps, core_ids)
    LAST_PROFILE[label] = br.exec_time_ns
    return br.results


def kernel(x, edge_index, W0, att_src0, att_dst0, b0, W1, att_src1, att_dst1, b1):
    x = np.asarray(x, np.float32)
    edge_index = np.asarray(edge_index)
    d = _dims_full()
    d["N_TAB"] = d["N"]
    K_LO, K_HI, idx_arrs = _prep_edges(edge_index, d)
    d["K_LO"], d["K_HI"] = K_LO, K_HI

    key = (K_LO, K_HI)
    if key not in _cache:
        _cache[key] = (
            build_phase_a(d),
            build_layer0_edges(d),
            build_layer1_edges(d),
        )
    nc1, nc2, nc3 = _cache[key]

    N, NLOC, NP = d["N"], d["NLOC"], d["NLOC_PAD"]
    eye = np.eye(128, dtype=np.float32)
    iota = _bf16(
        np.tile(np.arange(128, dtype=np.float32)[None, None, :], (128, CPC, 1))
    )
    A0 = _build_A0(np.asarray(att_src0), np.asarray(att_dst0))
    W1f = np.asarray(W1, np.float32)
    W1b = _bf16(W1f)
    A1e = _bf16(
        W1f
        @ np.stack(
            [np.asarray(att_src1).ravel(), np.asarray(att_dst1).ravel()], axis=1
        ).astype(np.float32)
    )
    b0r = np.tile(np.asarray(b0, np.float32)[None, :], (128, 1))
    b1r = np.tile(np.asarray(b1, np.float32)[None, :], (128, 1))
    core_ids = list(range(NCORES))

    in1 = []
    for c in range(NCORES):
        xs = x[c * NLOC : (c + 1) * NLOC]
        xT = np.zeros((d["F_IN"], NP), np.float32)
        xT[:, :NLOC] = xs.T
        in1.append(dict(xT=xT, W0=np.asarray(W0, np.float32), A0=A0, eye=eye))
    r1 = _run(nc1, in1, core_ids, "l1")
    table0 = np.concatenate([r1[c]["table0"][:NLOC] for c in range(NCORES)], axis=0)
    atab0 = np.concatenate([r1[c]["atab0"][:NLOC] for c in range(NCORES)], axis=0)
    astab0, adtab0 = atab0[:, 0:4], atab0[:, 4:8]

    def edge_inputs(c, astab, adtab, extra):
        ins = dict(extra, iota=iota)
        for s in range(2):
            gw, rrT, sg, dg = idx_arrs[c][s]
            ins[f"g{s}"] = gw
            ins[f"r{s}"] = _bf16(rrT)
            ins[f"as{s}"] = np.ascontiguousarray(
                astab[sg].transpose(1, 0, 2)
            ).astype(np.float32)
            ins[f"ad{s}"] = np.ascontiguousarray(
                adtab[dg].transpose(1, 0, 2)
            ).astype(np.float32)
        return ins

    in2 = [
        edge_inputs(
            c,
            astab0,
            adtab0,
            dict(table0=table0, W1b=W1b, A1e=A1e, b0r=b0r, eye=eye),
        )
        for c in range(NCORES)
    ]
    r2 = _run(nc2, in2, core_ids, "l2")
    table1 = np.concatenate([r2[c]["table1"][:NLOC] for c in range(NCORES)], axis=0)
    atab1 = np.concatenate([r2[c]["atab1"][:NLOC] for c in range(NCORES)], axis=0)
    astab1, adtab1 = atab1[:, 0:1], atab1[:, 1:2]

    in3 = [
        edge_inputs(c, astab1, adtab1, dict(table1=table1, b1r=b1r, eye=eye))
        for c in range(NCORES)
    ]
    r3 = _run(nc3, in3, core_ids, "l3")
    out = np.concatenate([r3[c]["out"][:NLOC] for c in range(NCORES)], axis=0)
    return out


# revision 18
# speedup vs baseline: 1.0960x; 1.0960x over previous
"""Two-layer GAT (PyG-style GATConv x2) on 8 Trainium2 NeuronCores — v2.

Sharding: nodes (and incident edges, by destination) sharded across 8
cores; small weights replicated. Per-edge source rows fetched via SWDGE
dma_gather from a row-major node table in HBM (bf16 h rows for layer 0,
fp32 h1 rows for layer 1 — both exactly 512 B / 256 B per row, the
gather's minimum-efficient granularity). Edges are dst-sorted and
grouped per 128-row dst tile; each 128-edge chunk is segment-reduced
with a one-hot matmul into PSUM.

v2 changes vs v1 (2.14 ms):
 - Node tables carry ONLY features. Attention alphas are emitted as
   separate per-node fp32 tables; the host expands them per edge (pure
   fancy-indexing, same category as v1's alpha_dst expansion) so logits
   are assembled on-chip from two sequentially-streamed fp32 inputs.
   Layer-0 gather rows shrink 768 B -> 512 B.
 - Softmax denominators accumulate via a second tiny matmul per chunk
   (lhsT = exp-weights) instead of embedding weight columns in the
   gathered payload — kills v1's pathological strided DVE copy
   (~380 us/launch).
 - All PSUM->SBUF evacuation copies moved to the idle Scalar engine.
 - Gather calls: 16 chunks per call, striped round-robin across all 4
   SWDGE queues.
"""

import os

import numpy as np

import concourse.bacc as bacc
import concourse.mybir as mybir
from concourse import tile
from concourse.bass_utils import run_bass_kernel_spmd

fp32 = mybir.dt.float32
bf16 = mybir.dt.bfloat16
i16 = mybir.dt.int16
Alu = mybir.AluOpType
Act = mybir.ActivationFunctionType

NCORES = 8
NEG_SLOPE = 0.2
EPS = 1e-16
CPC = 8  # 128-edge chunks per gather call (1024 idx — SWDGE ring limit)


def _dims_full():
    return dict(
        N=50000,
        NLOC=6250,
        NLOC_PAD=6272,
        F_IN=256,
        HID=256,
        H=4,
        DH=64,
        C_OUT=64,
        ELEM0=256,  # bf16 h row -> 512 B
        ELEM1=64,  # fp32 h1 row -> 256 B
        SPLIT=32768,  # int16 gather-index split point
    )


# ---------------------------------------------------------------- launch 1


def build_phase_a(d):
    """Per core: h0 = x_shard @ W0 -> bf16 table0 rows; alphas -> fp32 atab0."""
    nc = bacc.Bacc(None, target_bir_lowering=False, debug=False, num_swdge_queues=4)
    NP, F, HID = d["NLOC_PAD"], d["F_IN"], d["HID"]
    assert F == 256 and HID == 256

    xT = nc.dram_tensor("xT", [F, NP], fp32, kind="ExternalInput")
    W0 = nc.dram_tensor("W0", [F, HID], fp32, kind="ExternalInput")
    A0 = nc.dram_tensor("A0", [HID, 8], fp32, kind="ExternalInput")
    eye = nc.dram_tensor("eye", [128, 128], fp32, kind="ExternalInput")
    table0 = nc.dram_tensor("table0", [NP, 256], bf16, kind="ExternalOutput")
    atab0 = nc.dram_tensor("atab0", [NP, 8], fp32, kind="ExternalOutput")

    TW = 512
    n_t = (NP + TW - 1) // TW

    with tile.TileContext(nc) as tc:
        with (
            tc.tile_pool(name="const", bufs=1) as cpool,
            tc.tile_pool(name="work", bufs=3) as pool,
            tc.tile_pool(name="psum", bufs=1, space="PSUM") as pp,
            tc.tile_pool(name="psum1", bufs=2, space="PSUM") as pp1,
        ):
            w0_sb = [
                cpool.tile([128, HID], fp32, tag=f"w0_{k}", name=f"w0_{k}")
                for k in range(2)
            ]
            a0_sb = [
                cpool.tile([128, 8], fp32, tag=f"a0_{k}", name=f"a0_{k}")
                for k in range(2)
            ]
            eye_sb = cpool.tile([128, 128], fp32)
            for k in range(2):
                nc.sync.dma_start(w0_sb[k][:], W0[128 * k : 128 * (k + 1), :])
                nc.sync.dma_start(a0_sb[k][:], A0[128 * k : 128 * (k + 1), :])
            nc.sync.dma_start(eye_sb[:], eye[:])

            for t in range(n_t):
                c0 = t * TW
                cw = min(TW, NP - c0)
                xt = [
                    pool.tile([128, TW], fp32, tag=f"xt{k}", name=f"xt{k}")
                    for k in range(2)
                ]
                for k in range(2):
                    nc.sync.dma_start(
                        xt[k][:, :cw], xT[128 * k : 128 * (k + 1), c0 : c0 + cw]
                    )
                hT = [
                    pool.tile([128, TW], fp32, tag=f"ht{m}", name=f"ht{m}")
                    for m in range(2)
                ]
                for m in range(2):
                    ps = pp.tile([128, TW], fp32, tag=f"ps{m}", name=f"ps{m}")
                    for k in range(2):
                        nc.tensor.matmul(
                            ps[:, :cw],
                            w0_sb[k][:, 128 * m : 128 * (m + 1)],
                            xt[k][:, :cw],
                            start=(k == 0),
                            stop=(k == 1),
                        )
                    nc.scalar.activation(hT[m][:, :cw], ps[:, :cw], Act.Copy)

                nq = (cw + 127) // 128
                for q in range(nq):
                    q0 = q * 128
                    qw = min(128, cw - q0)
                    pa = pp1.tile([128, 8], fp32, tag="pa")
                    for k in range(2):
                        nc.tensor.matmul(
                            pa[:qw, :],
                            hT[k][:, q0 : q0 + qw],
                            a0_sb[k][:],
                            start=(k == 0),
                            stop=(k == 1),
                        )
                    R = pool.tile([128, 256], bf16, tag="rows")
                    for m in range(2):
                        pt = pp1.tile([128, 128], fp32, tag=f"pt{m}", name=f"pt{m}")
                        nc.tensor.transpose(
                            pt[:qw, :], hT[m][:, q0 : q0 + qw], eye_sb[:]
                        )
                        nc.scalar.activation(
                            R[:qw, 128 * m : 128 * (m + 1)], pt[:qw, :], Act.Copy
                        )
                    paS = pool.tile([128, 8], fp32, tag="paS")
                    nc.scalar.activation(paS[:qw, :], pa[:qw, :], Act.Copy)
                    r0 = c0 + q0
                    nc.sync.dma_start(table0[r0 : r0 + qw, :], R[:qw, :])
                    nc.sync.dma_start(atab0[r0 : r0 + qw, :], paS[:qw, :])
    nc.compile()
    return nc


# ------------------------------------------------------------ edge machinery


def _edge_pass(nc, tc, d, table, streams_dram, elem, gdt, nfeat, nhead, fin, pp):
    """Dst-sorted edge pass. Per gather call (CPC chunks of 128 edges):
    fetch source rows (SWDGE gather, round-robin over the 4 queues),
    assemble logits from the two host-expanded per-edge alpha streams,
    leaky-relu + exp, build one-hot via is_equal, weight the payload.
    Per chunk: main one-hot matmul accumulates the weighted messages per
    dst tile; a second tiny matmul (lhsT = exp-weights) accumulates the
    softmax denominators as dnT [nhead, 128]."""
    NP, SPLIT, NROWS = d["NLOC_PAD"], d["SPLIT"], d["N_TAB"]
    K_LO, K_HI = d["K_LO"], d["K_HI"]
    NT = NP // 128
    qcnt = [0]

    with (
        tc.tile_pool(name="eidx", bufs=1) as ipool,
        tc.tile_pool(name="edge", bufs=3) as pool,
    ):
        iota_sb = ipool.tile([128, CPC, 128], bf16)
        nc.sync.dma_start(iota_sb[:], d["iota_dram"][:])
        streams = []
        for s, K in ((0, K_LO), (1, K_HI)):
            gi_d, rr_d, as_d, ad_d = streams_dram[s]
            nch = NT * K
            gi = ipool.tile([128, nch * 8], i16, name=f"gi{s}")
            rr = ipool.tile([128, nch], bf16, name=f"rr{s}")
            asx = ipool.tile([128, nch, nhead], fp32, name=f"as{s}")
            adx = ipool.tile([128, nch, nhead], fp32, name=f"ad{s}")
            nc.sync.dma_start(gi[:], gi_d[:])
            nc.sync.dma_start(rr[:], rr_d[:])
            nc.sync.dma_start(asx[:], as_d[:])
            nc.sync.dma_start(adx[:], ad_d[:])
            base = table[0:SPLIT, :] if s == 0 else table[SPLIT:NROWS, :]
            streams.append(
                dict(gi=gi, rr=rr, asx=asx, adx=adx, K=K, base=base, ncalls=0,
                     tiles={})
            )

        def emit_call(st, call):
            c0 = call * CPC
            nch = min(CPC, NT * st["K"] - c0)
            ne = nch * 128
            G = pool.tile([128, CPC, elem], gdt, tag="G", name="G", bufs=4)
            OH = pool.tile([128, CPC, 128], bf16, tag="OH", name="OH", bufs=4)
            nc.gpsimd.dma_gather(
                G[:, :nch, :],
                st["base"],
                st["gi"][:, c0 * 8 : c0 * 8 + ne // 16],
                ne,
                ne,
                elem,
                queue_num=qcnt[0] % 4,
            )
            qcnt[0] += 1
            ew = pool.tile([128, CPC, nhead], fp32, tag="ew", name="ew", bufs=4)
            nc.vector.tensor_tensor(
                ew[:, :nch, :],
                st["asx"][:, c0 : c0 + nch, :],
                st["adx"][:, c0 : c0 + nch, :],
                op=Alu.add,
            )
            nc.vector.scalar_tensor_tensor(
                ew[:, :nch, :],
                ew[:, :nch, :],
                NEG_SLOPE,
                ew[:, :nch, :],
                op0=Alu.mult,
                op1=Alu.max,
            )
            ewb = pool.tile([128, CPC, nhead], bf16, tag="ewb", name="ewb", bufs=4)
            nc.scalar.activation(ewb[:, :nch, :], ew[:, :nch, :], Act.Exp)
            rb = st["rr"][:, c0 : c0 + nch].unsqueeze(2).broadcast_to(
                [128, nch, 128]
            )
            nc.vector.tensor_tensor(
                OH[:, :nch, :], rb, iota_sb[:, :nch, :], op=Alu.is_equal
            )
            if gdt == bf16:
                gm = G[:, :nch, :].rearrange("p c (h e) -> p c h e", h=nhead)
                wb = (
                    ewb[:, :nch, :]
                    .unsqueeze(3)
                    .broadcast_to([128, nch, nhead, nfeat // nhead])
                )
                nc.vector.tensor_tensor(gm, gm, wb, op=Alu.mult)
                Gw = G
            else:
                Gw = pool.tile([128, CPC, elem], bf16, tag="Gw", name="Gw", bufs=4)
                wb = ewb[:, :nch, :].broadcast_to([128, nch, nfeat])
                nc.vector.tensor_tensor(Gw[:, :nch, :], G[:, :nch, :], wb,
                                        op=Alu.mult)
            return Gw, OH, ewb

        for t in range(NT):
            ps = pp.tile([128, nfeat], fp32, tag="ps", name="ps")
            dn = pp.tile([128, nhead], fp32, tag="dn", name="dn")
            first = True
            for st in streams:
                K = st["K"]
                for k in range(K):
                    c = t * K + k
                    call, cin = c // CPC, c % CPC
                    if call >= st["ncalls"]:
                        st["tiles"][call] = emit_call(st, call)
                        st["ncalls"] = call + 1
                        st["tiles"].pop(call - 3, None)
                    Gw, OH, ewb = st["tiles"][call]
                    last = st is streams[1] and k == K - 1
                    nc.tensor.matmul(
                        ps[:],
                        OH[:, cin, :],
                        Gw[:, cin, 0:nfeat],
                        start=first,
                        stop=last,
                        skip_group_check=True,
                    )
                    nc.tensor.matmul(
                        dn[:],
                        OH[:, cin, :],
                        ewb[:, cin, :],
                        start=first,
                        stop=last,
                        skip_group_check=True,
                    )
                    first = False
            fin(t, ps, dn)


# ---------------------------------------------------------------- launch 2


def build_layer0_edges(d):
    """Layer-0 edge pass with fused finalize (softmax-div + bias + ELU),
    then h1 = h0' @ W1 -> fp32 table1 rows + fp32 atab1."""
    nc = bacc.Bacc(None, target_bir_lowering=False, debug=False, num_swdge_queues=4)
    NP = d["NLOC_PAD"]
    HID, C_OUT, H, DH = d["HID"], d["C_OUT"], d["H"], d["DH"]
    NT = NP // 128

    table0 = nc.dram_tensor("table0", [d["N_TAB"], 256], bf16, kind="ExternalInput")
    sd = []
    for s, K in ((0, d["K_LO"]), (1, d["K_HI"])):
        nch = NT * K
        sd.append(
            (
                nc.dram_tensor(f"g{s}", [128, nch * 8], i16, kind="ExternalInput"),
                nc.dram_tensor(f"r{s}", [128, nch], bf16, kind="ExternalInput"),
                nc.dram_tensor(f"as{s}", [128, nch, H], fp32, kind="ExternalInput"),
                nc.dram_tensor(f"ad{s}", [128, nch, H], fp32, kind="ExternalInput"),
            )
        )
    iota = nc.dram_tensor("iota", [128, CPC, 128], bf16, kind="ExternalInput")
    W1b = nc.dram_tensor("W1b", [HID, C_OUT], bf16, kind="ExternalInput")
    A1e = nc.dram_tensor("A1e", [HID, 2], bf16, kind="ExternalInput")
    b0r = nc.dram_tensor("b0r", [128, HID], fp32, kind="ExternalInput")
    eye = nc.dram_tensor("eye", [128, 128], fp32, kind="ExternalInput")
    table1 = nc.dram_tensor("table1", [NP, 64], fp32, kind="ExternalOutput")
    atab1 = nc.dram_tensor("atab1", [NP, 2], fp32, kind="ExternalOutput")
    d = dict(d, iota_dram=iota)

    with tile.TileContext(nc) as tc:
        with (
            tc.tile_pool(name="fconst", bufs=1) as cpool,
            tc.tile_pool(name="fin", bufs=3) as pool,
            tc.tile_pool(name="h0all", bufs=1) as hpool,
            tc.tile_pool(name="epsum", bufs=2, space="PSUM") as pp,
        ):
            b0_sb = cpool.tile([128, HID], fp32)
            nc.sync.dma_start(b0_sb[:], b0r[:])
            eye_sb = cpool.tile([128, 128], fp32)
            nc.sync.dma_start(eye_sb[:], eye[:])
            H0 = hpool.tile([128, NT, HID], fp32)

            def fin0(t, ps, dn):
                dnS = pool.tile([128, H], fp32, tag="dnS", name="dnS")
                nc.vector.tensor_scalar_add(dnS[:], dn[:], EPS)
                recB = pool.tile([128, H], fp32, tag="recB", name="recB")
                nc.vector.reciprocal(recB[:], dnS[:])
                f4 = ps[:, 0:HID].rearrange("p (h e) -> p h e", h=H)
                rb = recB[:].unsqueeze(2).broadcast_to([128, H, DH])
                hrow = H0[:, t, :]
                nc.vector.tensor_tensor(
                    hrow.rearrange("p (h e) -> p h e", h=H), f4, rb, op=Alu.mult
                )
                nc.vector.tensor_tensor(hrow, hrow, b0_sb[:], op=Alu.add)
                tn = pool.tile([128, HID], fp32, tag="tn", name="tn")
                nc.vector.tensor_scalar_min(tn[:], hrow, 0.0)
                nc.scalar.activation(tn[:], tn[:], Act.Exp)
                tp = pool.tile([128, HID], fp32, tag="tp", name="tp")
                nc.vector.tensor_scalar_max(tp[:], hrow, 0.0)
                nc.vector.scalar_tensor_tensor(
                    hrow, tn[:], -1.0, tp[:], op0=Alu.add, op1=Alu.add
                )

            _edge_pass(nc, tc, d, table0, sd, 256, bf16, HID, H, fin0, pp)

            with (
                tc.tile_pool(name="tb1", bufs=3) as tpool,
                tc.tile_pool(name="tb1psum", bufs=1, space="PSUM") as pp2,
            ):
                w1_sb = [
                    cpool.tile([128, C_OUT], bf16, tag=f"w1_{k}", name=f"w1_{k}")
                    for k in range(2)
                ]
                a1e_sb = [
                    cpool.tile([128, 2], bf16, tag=f"a1e_{k}", name=f"a1e_{k}")
                    for k in range(2)
                ]
                for k in range(2):
                    nc.sync.dma_start(w1_sb[k][:], W1b[128 * k : 128 * (k + 1), :])
                    nc.sync.dma_start(a1e_sb[k][:], A1e[128 * k : 128 * (k + 1), :])

                for r in range(NT):
                    h0T = [
                        tpool.tile([128, 128], bf16, tag=f"h0T{k}", name=f"h0T{k}")
                        for k in range(2)
                    ]
                    for k in range(2):
                        pt = pp2.tile([128, 128], fp32, tag="pt", name="pt")
                        nc.tensor.transpose(
                            pt[:], H0[:, r, 128 * k : 128 * (k + 1)], eye_sb[:]
                        )
                        nc.scalar.activation(h0T[k][:], pt[:], Act.Copy)
                    pr1 = pp2.tile([128, C_OUT], fp32, tag="pr1", name="pr1")
                    pr2 = pp2.tile([128, 2], fp32, tag="pr2", name="pr2")
                    for k in range(2):
                        nc.tensor.matmul(
                            pr1[:],
                            h0T[k][:],
                            w1_sb[k][:],
                            start=(k == 0),
                            stop=(k == 1),
                            skip_group_check=True,
                        )
                        nc.tensor.matmul(
                            pr2[:],
                            h0T[k][:],
                            a1e_sb[k][:],
                            start=(k == 0),
                            stop=(k == 1),
                            skip_group_check=True,
                        )
                    R1 = tpool.tile([128, C_OUT], fp32, tag="R1", name="R1")
                    nc.scalar.activation(R1[:], pr1[:], Act.Copy)
                    palS = tpool.tile([128, 2], fp32, tag="palS", name="palS")
                    nc.scalar.activation(palS[:], pr2[:], Act.Copy)
                    nc.sync.dma_start(table1[128 * r : 128 * (r + 1), :], R1[:])
                    nc.sync.dma_start(atab1[128 * r : 128 * (r + 1), :], palS[:])
    nc.compile()
    return nc


# ---------------------------------------------------------------- launch 3


def build_layer1_edges(d):
    """Layer-1 edge pass with fused finalize -> output shard."""
    nc = bacc.Bacc(None, target_bir_lowering=False, debug=False, num_swdge_queues=4)
    NP, C_OUT = d["NLOC_PAD"], d["C_OUT"]
    NT = NP // 128

    table1 = nc.dram_tensor("table1", [d["N_TAB"], 64], fp32, kind="ExternalInput")
    sd = []
    for s, K in ((0, d["K_LO"]), (1, d["K_HI"])):
        nch = NT * K
        sd.append(
            (
                nc.dram_tensor(f"g{s}", [128, nch * 8], i16, kind="ExternalInput"),
                nc.dram_tensor(f"r{s}", [128, nch], bf16, kind="ExternalInput"),
                nc.dram_tensor(f"as{s}", [128, nch, 1], fp32, kind="ExternalInput"),
                nc.dram_tensor(f"ad{s}", [128, nch, 1], fp32, kind="ExternalInput"),
            )
        )
    iota = nc.dram_tensor("iota", [128, CPC, 128], bf16, kind="ExternalInput")
    eye = nc.dram_tensor("eye", [128, 128], fp32, kind="ExternalInput")
    b1r = nc.dram_tensor("b1r", [128, C_OUT], fp32, kind="ExternalInput")
    out = nc.dram_tensor("out", [NP, C_OUT], fp32, kind="ExternalOutput")
    d = dict(d, iota_dram=iota)

    with tile.TileContext(nc) as tc:
        with (
            tc.tile_pool(name="oconst", bufs=1) as cpool,
            tc.tile_pool(name="ofin", bufs=3) as pool,
            tc.tile_pool(name="epsum", bufs=2, space="PSUM") as pp,
        ):
            b1_sb = cpool.tile([128, C_OUT], fp32)
            nc.sync.dma_start(b1_sb[:], b1r[:])

            def fin1(t, ps, dn):
                dnS = pool.tile([128, 1], fp32, tag="dnS", name="dnS")
                nc.vector.tensor_scalar_add(dnS[:], dn[:], EPS)
                recB = pool.tile([128, 1], fp32, tag="recB", name="recB")
                nc.vector.reciprocal(recB[:], dnS[:])
                O = pool.tile([128, C_OUT], fp32, tag="O", name="O")
                rb = recB[:].broadcast_to([128, C_OUT])
                nc.vector.tensor_tensor(O[:], ps[:, 0:C_OUT], rb, op=Alu.mult)
                nc.vector.tensor_tensor(O[:], O[:], b1_sb[:], op=Alu.add)
                nc.sync.dma_start(out[128 * t : 128 * (t + 1), :], O[:])

            _edge_pass(nc, tc, d, table1, sd, 64, fp32, C_OUT, 1, fin1, pp)
    nc.compile()
    return nc


# ------------------------------------------------------------ host plumbing


def _wrap_idx(idx):
    """idx[j] -> [j%16, j//16], replicated across the 8 q7 core groups."""
    a = idx.reshape(-1, 16).T.astype(np.int16)
    return np.tile(a, (8, 1))


def _prep_edges(edge_index, d):
    """Partition edges by dst shard; per core split by src < SPLIT (int16
    gather range), group by 128-row dst tile (sorted by dst), pad each
    (tile, stream) segment to the global max chunk count K_LO / K_HI.

    Returns per core, per stream: (wrapped_idx, rrT, src_global, dst_global)
    where src/dst_global are [NT*K, 128] int64 node ids (0 on pad slots)."""
    N, NLOC, NP = d["N"], d["NLOC"], d["NLOC_PAD"]
    SPLIT = d["SPLIT"]
    NT = NP // 128
    src = np.concatenate([edge_index[0], np.arange(N, dtype=np.int64)])
    dst = np.concatenate([edge_index[1], np.arange(N, dtype=np.int64)])
    core = dst // NLOC
    per_core = []
    kmax = [1, 1]
    for c in range(NCORES):
        m = core == c
        s, t = src[m], dst[m] - c * NLOC
        order = np.argsort(t, kind="stable")
        s, t = s[order], t[order]
        lo = s < SPLIT
        segs = []
        for sm, base in ((lo, 0), (~lo, SPLIT)):
            ss, tt = s[sm], t[sm]
            counts = np.bincount(tt // 128, minlength=NT)
            segs.append((ss, tt, counts, base))
        per_core.append(segs)
        for si in range(2):
            kmax[si] = max(kmax[si], int(np.ceil(per_core[c][si][2].max() / 128)))
    K_LO, K_HI = kmax
    res = []
    for c in range(NCORES):
        arrs = []
        for si, K in ((0, K_LO), (1, K_HI)):
            ss, tt, counts, base = per_core[c][si]
            g = np.zeros((NT, K * 128), np.int64)
            sg = np.zeros((NT, K * 128), np.int64)
            dg = np.zeros((NT, K * 128), np.int64)
            rr = np.full((NT, K * 128), -1.0, np.float32)
            offs = np.concatenate([[0], np.cumsum(counts)])
            for tl in range(NT):
                n = counts[tl]
                g[tl, :n] = ss[offs[tl] : offs[tl] + n] - base
                sg[tl, :n] = ss[offs[tl] : offs[tl] + n]
                dg[tl, :n] = tt[offs[tl] : offs[tl] + n] + c * NLOC
                rr[tl, :n] = (tt[offs[tl] : offs[tl] + n] - 128 * tl).astype(
                    np.float32
                )
            arrs.append(
                (
                    _wrap_idx(g.ravel()),
                    np.ascontiguousarray(rr.reshape(NT * K, 128).T),
                    sg.reshape(NT * K, 128),
                    dg.reshape(NT * K, 128),
                )
            )
        res.append(arrs)
    return K_LO, K_HI, res


def _build_A0(att_src, att_dst):
    H, DH = att_src.shape
    A = np.zeros((H * DH, 2 * H), np.float32)
    for h in range(H):
        A[h * DH : (h + 1) * DH, h] = att_src[h]
        A[h * DH : (h + 1) * DH, H + h] = att_dst[h]
    return A


def _bf16(a):
    import ml_dtypes

    return a.astype(ml_dtypes.bfloat16)


_cache = {}
LAST_PROFILE = {}


def _run(nc, in_maps, core_ids, label):
    trace = bool(int(os.environ.get("GAT_PROFILE", "0")))
    if trace:
        try:
            import sys

            import profile_hook

            profile_hook.install()
            import concourse.bass_utils as bu

            bu.upload_artifacts = lambda tmpdir: "local://skipped"
            br = run_bass_kernel_spmd(nc, in_maps, core_ids, trace=True)
            LAST_PROFILE[label] = br.exec_time_ns
            return br.results
        except Exception as e:  # fall back to untraced
            print(f"traced run failed ({e!r}); untraced retry", file=sys.stderr)
    br = run_bass_kernel_spmd(nc, in_maps, core_ids)
    LAST_PROFILE[label] = br.exec_time_ns
    return br.results


def kernel(x, edge_index, W0, att_src0, att_dst0, b0, W1, att_src1, att_dst1, b1):
    x = np.asarray(x, np.float32)
    edge_index = np.asarray(edge_index)
    d = _dims_full()
    d["N_TAB"] = d["N"]
    K_LO, K_HI, idx_arrs = _prep_edges(edge_index, d)
    d["K_LO"], d["K_HI"] = K_LO, K_HI

    key = (K_LO, K_HI)
    if key not in _cache:
        _cache[key] = (
            build_phase_a(d),
            build_layer0_edges(d),
            build_layer1_edges(d),
        )
    nc1, nc2, nc3 = _cache[key]

    N, NLOC, NP = d["N"], d["NLOC"], d["NLOC_PAD"]
    eye = np.eye(128, dtype=np.float32)
    iota = _bf16(
        np.tile(np.arange(128, dtype=np.float32)[None, None, :], (128, CPC, 1))
    )
    A0 = _build_A0(np.asarray(att_src0), np.asarray(att_dst0))
    W1f = np.asarray(W1, np.float32)
    W1b = _bf16(W1f)
    A1e = _bf16(
        W1f
        @ np.stack(
            [np.asarray(att_src1).ravel(), np.asarray(att_dst1).ravel()], axis=1
        ).astype(np.float32)
    )
    b0r = np.tile(np.asarray(b0, np.float32)[None, :], (128, 1))
    b1r = np.tile(np.asarray(b1, np.float32)[None, :], (128, 1))
    core_ids = list(range(NCORES))

    in1 = []
    for c in range(NCORES):
        xs = x[c * NLOC : (c + 1) * NLOC]
        xT = np.zeros((d["F_IN"], NP), np.float32)
        xT[:, :NLOC] = xs.T
        in1.append(dict(xT=xT, W0=np.asarray(W0, np.float32), A0=A0, eye=eye))
    r1 = _run(nc1, in1, core_ids, "l1")
    table0 = np.concatenate([r1[c]["table0"][:NLOC] for c in range(NCORES)], axis=0)
    atab0 = np.concatenate([r1[c]["atab0"][:NLOC] for c in range(NCORES)], axis=0)
    astab0, adtab0 = atab0[:, 0:4], atab0[:, 4:8]

    def edge_inputs(c, astab, adtab, extra):
        ins = dict(extra, iota=iota)
        for s in range(2):
            gw, rrT, sg, dg = idx_arrs[c][s]
            ins[f"g{s}"] = gw
            ins[f"r{s}"] = _bf16(rrT)
            ins[f"as{s}"] = np.ascontiguousarray(
                astab[sg].transpose(1, 0, 2)
            ).astype(np.float32)
            ins[f"ad{s}"] = np.ascontiguousarray(
                adtab[dg].transpose(1, 0, 2)
            ).astype(np.float32)
        return ins

    in2 = [
        edge_inputs(
            c,
            astab0,
            adtab0,
            dict(table0=table0, W1b=W1b, A1e=A1e, b0r=b0r, eye=eye),
        )
        for c in range(NCORES)
    ]
    r2 = _run(nc2, in2, core_ids, "l2")
    table1 = np.concatenate([r2[c]["table1"][:NLOC] for c in range(NCORES)], axis=0)
    atab1 = np.concatenate([r2[c]["atab1"][:NLOC] for c in range(NCORES)], axis=0)
    astab1, adtab1 = atab1[:, 0:1], atab1[:, 1:2]

    in3 = [
        edge_inputs(c, astab1, adtab1, dict(table1=table1, b1r=b1r, eye=eye))
        for c in range(NCORES)
    ]
    r3 = _run(nc3, in3, core_ids, "l3")
    out = np.concatenate([r3[c]["out"][:NLOC] for c in range(NCORES)], axis=0)
    return out


# revision 20
# speedup vs baseline: 1.2805x; 1.1684x over previous
"""Two-layer GAT (PyG-style GATConv x2) on 8 Trainium2 NeuronCores — v2.

Sharding: nodes (and incident edges, by destination) sharded across 8
cores; small weights replicated. Per-edge source rows fetched via SWDGE
dma_gather from a row-major node table in HBM (bf16 h rows for layer 0,
fp32 h1 rows for layer 1 — both exactly 512 B / 256 B per row, the
gather's minimum-efficient granularity). Edges are dst-sorted and
grouped per 128-row dst tile; each 128-edge chunk is segment-reduced
with a one-hot matmul into PSUM.

v2 changes vs v1 (2.14 ms):
 - Node tables carry ONLY features. Attention alphas are emitted as
   separate per-node fp32 tables; the host expands them per edge (pure
   fancy-indexing, same category as v1's alpha_dst expansion) so logits
   are assembled on-chip from two sequentially-streamed fp32 inputs.
   Layer-0 gather rows shrink 768 B -> 512 B.
 - Softmax denominators accumulate via a second tiny matmul per chunk
   (lhsT = exp-weights) instead of embedding weight columns in the
   gathered payload — kills v1's pathological strided DVE copy
   (~380 us/launch).
 - All PSUM->SBUF evacuation copies moved to the idle Scalar engine.
 - Gather calls: 16 chunks per call, striped round-robin across all 4
   SWDGE queues.
"""

import os

import numpy as np

import concourse.bacc as bacc
import concourse.mybir as mybir
from concourse import tile
from concourse.bass_utils import run_bass_kernel_spmd

fp32 = mybir.dt.float32
bf16 = mybir.dt.bfloat16
i16 = mybir.dt.int16
Alu = mybir.AluOpType
Act = mybir.ActivationFunctionType

NCORES = 8
NEG_SLOPE = 0.2
EPS = 1e-16
CPC = 8  # 128-edge chunks per gather call (1024 idx — SWDGE ring limit)


def _dims_full():
    return dict(
        N=50000,
        NLOC=6250,
        NLOC_PAD=6272,
        F_IN=256,
        HID=256,
        H=4,
        DH=64,
        C_OUT=64,
        ELEM0=256,  # bf16 h row -> 512 B
        ELEM1=64,  # fp32 h1 row -> 256 B
        SPLIT=32768,  # int16 gather-index split point
    )


# ---------------------------------------------------------------- launch 1


def build_phase_a(d):
    """Per core: h0 = x_shard @ W0 -> bf16 table0 rows; alphas -> fp32 atab0."""
    nc = bacc.Bacc(None, target_bir_lowering=False, debug=False, num_swdge_queues=4)
    NP, F, HID = d["NLOC_PAD"], d["F_IN"], d["HID"]
    assert F == 256 and HID == 256

    xT = nc.dram_tensor("xT", [F, NP], fp32, kind="ExternalInput")
    W0 = nc.dram_tensor("W0", [F, HID], fp32, kind="ExternalInput")
    A0 = nc.dram_tensor("A0", [HID, 8], fp32, kind="ExternalInput")
    eye = nc.dram_tensor("eye", [128, 128], fp32, kind="ExternalInput")
    table0 = nc.dram_tensor("table0", [NP, 256], bf16, kind="ExternalOutput")
    atab0 = nc.dram_tensor("atab0", [NP, 8], fp32, kind="ExternalOutput")

    TW = 512
    n_t = (NP + TW - 1) // TW

    with tile.TileContext(nc) as tc:
        with (
            tc.tile_pool(name="const", bufs=1) as cpool,
            tc.tile_pool(name="work", bufs=3) as pool,
            tc.tile_pool(name="psum", bufs=1, space="PSUM") as pp,
            tc.tile_pool(name="psum1", bufs=2, space="PSUM") as pp1,
        ):
            w0_sb = [
                cpool.tile([128, HID], fp32, tag=f"w0_{k}", name=f"w0_{k}")
                for k in range(2)
            ]
            a0_sb = [
                cpool.tile([128, 8], fp32, tag=f"a0_{k}", name=f"a0_{k}")
                for k in range(2)
            ]
            eye_sb = cpool.tile([128, 128], fp32)
            for k in range(2):
                nc.sync.dma_start(w0_sb[k][:], W0[128 * k : 128 * (k + 1), :])
                nc.sync.dma_start(a0_sb[k][:], A0[128 * k : 128 * (k + 1), :])
            nc.sync.dma_start(eye_sb[:], eye[:])

            for t in range(n_t):
                c0 = t * TW
                cw = min(TW, NP - c0)
                xt = [
                    pool.tile([128, TW], fp32, tag=f"xt{k}", name=f"xt{k}")
                    for k in range(2)
                ]
                for k in range(2):
                    nc.sync.dma_start(
                        xt[k][:, :cw], xT[128 * k : 128 * (k + 1), c0 : c0 + cw]
                    )
                hT = [
                    pool.tile([128, TW], fp32, tag=f"ht{m}", name=f"ht{m}")
                    for m in range(2)
                ]
                for m in range(2):
                    ps = pp.tile([128, TW], fp32, tag=f"ps{m}", name=f"ps{m}")
                    for k in range(2):
                        nc.tensor.matmul(
                            ps[:, :cw],
                            w0_sb[k][:, 128 * m : 128 * (m + 1)],
                            xt[k][:, :cw],
                            start=(k == 0),
                            stop=(k == 1),
                        )
                    nc.scalar.activation(hT[m][:, :cw], ps[:, :cw], Act.Copy)

                nq = (cw + 127) // 128
                for q in range(nq):
                    q0 = q * 128
                    qw = min(128, cw - q0)
                    pa = pp1.tile([128, 8], fp32, tag="pa")
                    for k in range(2):
                        nc.tensor.matmul(
                            pa[:qw, :],
                            hT[k][:, q0 : q0 + qw],
                            a0_sb[k][:],
                            start=(k == 0),
                            stop=(k == 1),
                        )
                    R = pool.tile([128, 256], bf16, tag="rows")
                    for m in range(2):
                        pt = pp1.tile([128, 128], fp32, tag=f"pt{m}", name=f"pt{m}")
                        nc.tensor.transpose(
                            pt[:qw, :], hT[m][:, q0 : q0 + qw], eye_sb[:]
                        )
                        nc.scalar.activation(
                            R[:qw, 128 * m : 128 * (m + 1)], pt[:qw, :], Act.Copy
                        )
                    paS = pool.tile([128, 8], fp32, tag="paS")
                    nc.scalar.activation(paS[:qw, :], pa[:qw, :], Act.Copy)
                    r0 = c0 + q0
                    nc.sync.dma_start(table0[r0 : r0 + qw, :], R[:qw, :])
                    nc.sync.dma_start(atab0[r0 : r0 + qw, :], paS[:qw, :])
    nc.compile()
    return nc


# ------------------------------------------------------------ edge machinery


def _edge_pass(nc, tc, d, table, streams_dram, elem, gdt, nfeat, nhead, fin, pp):
    """Dst-sorted edge pass. Per gather call (CPC chunks of 128 edges):
    fetch source rows (SWDGE gather, round-robin over the 4 queues),
    assemble logits from the two host-expanded per-edge alpha streams,
    leaky-relu + exp, build one-hot via is_equal, weight the payload.
    Per chunk: main one-hot matmul accumulates the weighted messages per
    dst tile; a second tiny matmul (lhsT = exp-weights) accumulates the
    softmax denominators as dnT [nhead, 128]."""
    NP, SPLIT, NROWS = d["NLOC_PAD"], d["SPLIT"], d["N_TAB"]
    K_LO, K_HI = d["K_LO"], d["K_HI"]
    NT = NP // 128
    qcnt = [0]

    with (
        tc.tile_pool(name="eidx", bufs=1) as ipool,
        tc.tile_pool(name="edge", bufs=3) as pool,
    ):
        iota_sb = ipool.tile([128, CPC, 128], bf16)
        nc.sync.dma_start(iota_sb[:], d["iota_dram"][:])
        streams = []
        for s, K in ((0, K_LO), (1, K_HI)):
            gi_d, rr_d, as_d, ad_d = streams_dram[s]
            nch = NT * K
            gi = ipool.tile([128, nch * 8], i16, name=f"gi{s}")
            rr = ipool.tile([128, nch], bf16, name=f"rr{s}")
            asx = ipool.tile([128, nch, nhead], fp32, name=f"as{s}")
            adx = ipool.tile([128, nch, nhead], fp32, name=f"ad{s}")
            nc.sync.dma_start(gi[:], gi_d[:])
            nc.sync.dma_start(rr[:], rr_d[:])
            nc.sync.dma_start(asx[:], as_d[:])
            nc.sync.dma_start(adx[:], ad_d[:])
            base = table[0:SPLIT, :] if s == 0 else table[SPLIT:NROWS, :]
            streams.append(
                dict(gi=gi, rr=rr, asx=asx, adx=adx, K=K, base=base, ncalls=0,
                     tiles={})
            )

        def emit_call(st, call):
            c0 = call * CPC
            nch = min(CPC, NT * st["K"] - c0)
            ne = nch * 128
            G = pool.tile([128, CPC, elem], gdt, tag="G", name="G", bufs=6)
            OH = pool.tile([128, CPC, 128], bf16, tag="OH", name="OH", bufs=6)
            nc.gpsimd.dma_gather(
                G[:, :nch, :],
                st["base"],
                st["gi"][:, c0 * 8 : c0 * 8 + ne // 16],
                ne,
                ne,
                elem,
                queue_num=qcnt[0] % 4,
            )
            qcnt[0] += 1
            ew = pool.tile([128, CPC, nhead], fp32, tag="ew", name="ew", bufs=6)
            nc.vector.tensor_tensor(
                ew[:, :nch, :],
                st["asx"][:, c0 : c0 + nch, :],
                st["adx"][:, c0 : c0 + nch, :],
                op=Alu.add,
            )
            nc.vector.scalar_tensor_tensor(
                ew[:, :nch, :],
                ew[:, :nch, :],
                NEG_SLOPE,
                ew[:, :nch, :],
                op0=Alu.mult,
                op1=Alu.max,
            )
            ewb = pool.tile([128, CPC, nhead], bf16, tag="ewb", name="ewb", bufs=6)
            nc.scalar.activation(ewb[:, :nch, :], ew[:, :nch, :], Act.Exp)
            rb = st["rr"][:, c0 : c0 + nch].unsqueeze(2).broadcast_to(
                [128, nch, 128]
            )
            nc.vector.tensor_tensor(
                OH[:, :nch, :], rb, iota_sb[:, :nch, :], op=Alu.is_equal
            )
            if gdt == bf16:
                gm = G[:, :nch, :].rearrange("p c (h e) -> p c h e", h=nhead)
                wb = (
                    ewb[:, :nch, :]
                    .unsqueeze(3)
                    .broadcast_to([128, nch, nhead, nfeat // nhead])
                )
                nc.vector.tensor_tensor(gm, gm, wb, op=Alu.mult)
                Gw = G
            else:
                Gw = pool.tile([128, CPC, elem], bf16, tag="Gw", name="Gw", bufs=6)
                wb = ewb[:, :nch, :].broadcast_to([128, nch, nfeat])
                nc.vector.tensor_tensor(Gw[:, :nch, :], G[:, :nch, :], wb,
                                        op=Alu.mult)
            return Gw, OH, ewb

        for t in range(NT):
            ps = pp.tile([128, nfeat], fp32, tag="ps", name="ps")
            dn = pp.tile([128, nhead], fp32, tag="dn", name="dn")
            first = True
            for st in streams:
                K = st["K"]
                for k in range(K):
                    c = t * K + k
                    call, cin = c // CPC, c % CPC
                    if call >= st["ncalls"]:
                        st["tiles"][call] = emit_call(st, call)
                        st["ncalls"] = call + 1
                        st["tiles"].pop(call - 5, None)
                    Gw, OH, ewb = st["tiles"][call]
                    last = st is streams[1] and k == K - 1
                    nc.tensor.matmul(
                        ps[:],
                        OH[:, cin, :],
                        Gw[:, cin, 0:nfeat],
                        start=first,
                        stop=last,
                        skip_group_check=True,
                    )
                    nc.tensor.matmul(
                        dn[:],
                        OH[:, cin, :],
                        ewb[:, cin, :],
                        start=first,
                        stop=last,
                        skip_group_check=True,
                    )
                    first = False
            fin(t, ps, dn)


# ---------------------------------------------------------------- launch 2


def build_layer0_edges(d):
    """Layer-0 edge pass with fused finalize (softmax-div + bias + ELU),
    then h1 = h0' @ W1 -> fp32 table1 rows + fp32 atab1."""
    nc = bacc.Bacc(None, target_bir_lowering=False, debug=False, num_swdge_queues=4)
    NP = d["NLOC_PAD"]
    HID, C_OUT, H, DH = d["HID"], d["C_OUT"], d["H"], d["DH"]
    NT = NP // 128

    table0 = nc.dram_tensor("table0", [d["N_TAB"], 256], bf16, kind="ExternalInput")
    sd = []
    for s, K in ((0, d["K_LO"]), (1, d["K_HI"])):
        nch = NT * K
        sd.append(
            (
                nc.dram_tensor(f"g{s}", [128, nch * 8], i16, kind="ExternalInput"),
                nc.dram_tensor(f"r{s}", [128, nch], bf16, kind="ExternalInput"),
                nc.dram_tensor(f"as{s}", [128, nch, H], fp32, kind="ExternalInput"),
                nc.dram_tensor(f"ad{s}", [128, nch, H], fp32, kind="ExternalInput"),
            )
        )
    iota = nc.dram_tensor("iota", [128, CPC, 128], bf16, kind="ExternalInput")
    W1b = nc.dram_tensor("W1b", [HID, C_OUT], bf16, kind="ExternalInput")
    A1e = nc.dram_tensor("A1e", [HID, 2], bf16, kind="ExternalInput")
    b0r = nc.dram_tensor("b0r", [128, HID], fp32, kind="ExternalInput")
    eye = nc.dram_tensor("eye", [128, 128], fp32, kind="ExternalInput")
    table1 = nc.dram_tensor("table1", [NP, 64], fp32, kind="ExternalOutput")
    atab1 = nc.dram_tensor("atab1", [NP, 2], fp32, kind="ExternalOutput")
    d = dict(d, iota_dram=iota)

    with tile.TileContext(nc) as tc:
        with (
            tc.tile_pool(name="fconst", bufs=1) as cpool,
            tc.tile_pool(name="fin", bufs=3) as pool,
            tc.tile_pool(name="h0all", bufs=1) as hpool,
            tc.tile_pool(name="epsum", bufs=2, space="PSUM") as pp,
        ):
            b0_sb = cpool.tile([128, HID], fp32)
            nc.sync.dma_start(b0_sb[:], b0r[:])
            eye_sb = cpool.tile([128, 128], fp32)
            nc.sync.dma_start(eye_sb[:], eye[:])
            H0 = hpool.tile([128, NT, HID], fp32)

            def fin0(t, ps, dn):
                dnS = pool.tile([128, H], fp32, tag="dnS", name="dnS")
                nc.vector.tensor_scalar_add(dnS[:], dn[:], EPS)
                recB = pool.tile([128, H], fp32, tag="recB", name="recB")
                nc.vector.reciprocal(recB[:], dnS[:])
                f4 = ps[:, 0:HID].rearrange("p (h e) -> p h e", h=H)
                rb = recB[:].unsqueeze(2).broadcast_to([128, H, DH])
                hrow = H0[:, t, :]
                nc.vector.tensor_tensor(
                    hrow.rearrange("p (h e) -> p h e", h=H), f4, rb, op=Alu.mult
                )
                nc.vector.tensor_tensor(hrow, hrow, b0_sb[:], op=Alu.add)
                # ELU on the Scalar engine: exp(min(x,0)) = Exp(-Relu(-x)),
                # max(x,0) = Relu(x); only the final combine stays on DVE.
                tn = pool.tile([128, HID], fp32, tag="tn", name="tn")
                nc.scalar.activation(tn[:], hrow, Act.Relu, scale=-1.0)
                nc.scalar.activation(tn[:], tn[:], Act.Exp, scale=-1.0)
                tp = pool.tile([128, HID], fp32, tag="tp", name="tp")
                nc.scalar.activation(tp[:], hrow, Act.Relu)
                nc.vector.scalar_tensor_tensor(
                    hrow, tn[:], -1.0, tp[:], op0=Alu.add, op1=Alu.add
                )

            _edge_pass(nc, tc, d, table0, sd, 256, bf16, HID, H, fin0, pp)

            with (
                tc.tile_pool(name="tb1", bufs=3) as tpool,
                tc.tile_pool(name="tb1psum", bufs=1, space="PSUM") as pp2,
            ):
                w1_sb = [
                    cpool.tile([128, C_OUT], bf16, tag=f"w1_{k}", name=f"w1_{k}")
                    for k in range(2)
                ]
                a1e_sb = [
                    cpool.tile([128, 2], bf16, tag=f"a1e_{k}", name=f"a1e_{k}")
                    for k in range(2)
                ]
                for k in range(2):
                    nc.sync.dma_start(w1_sb[k][:], W1b[128 * k : 128 * (k + 1), :])
                    nc.sync.dma_start(a1e_sb[k][:], A1e[128 * k : 128 * (k + 1), :])

                for r in range(NT):
                    h0T = [
                        tpool.tile([128, 128], bf16, tag=f"h0T{k}", name=f"h0T{k}")
                        for k in range(2)
                    ]
                    for k in range(2):
                        pt = pp2.tile([128, 128], fp32, tag="pt", name="pt")
                        nc.tensor.transpose(
                            pt[:], H0[:, r, 128 * k : 128 * (k + 1)], eye_sb[:]
                        )
                        nc.scalar.activation(h0T[k][:], pt[:], Act.Copy)
                    pr1 = pp2.tile([128, C_OUT], fp32, tag="pr1", name="pr1")
                    pr2 = pp2.tile([128, 2], fp32, tag="pr2", name="pr2")
                    for k in range(2):
                        nc.tensor.matmul(
                            pr1[:],
                            h0T[k][:],
                            w1_sb[k][:],
                            start=(k == 0),
                            stop=(k == 1),
                            skip_group_check=True,
                        )
                        nc.tensor.matmul(
                            pr2[:],
                            h0T[k][:],
                            a1e_sb[k][:],
                            start=(k == 0),
                            stop=(k == 1),
                            skip_group_check=True,
                        )
                    R1 = tpool.tile([128, C_OUT], fp32, tag="R1", name="R1")
                    nc.scalar.activation(R1[:], pr1[:], Act.Copy)
                    palS = tpool.tile([128, 2], fp32, tag="palS", name="palS")
                    nc.scalar.activation(palS[:], pr2[:], Act.Copy)
                    nc.sync.dma_start(table1[128 * r : 128 * (r + 1), :], R1[:])
                    nc.sync.dma_start(atab1[128 * r : 128 * (r + 1), :], palS[:])
    nc.compile()
    return nc


# ---------------------------------------------------------------- launch 3


def build_layer1_edges(d):
    """Layer-1 edge pass with fused finalize -> output shard."""
    nc = bacc.Bacc(None, target_bir_lowering=False, debug=False, num_swdge_queues=4)
    NP, C_OUT = d["NLOC_PAD"], d["C_OUT"]
    NT = NP // 128

    table1 = nc.dram_tensor("table1", [d["N_TAB"], 64], fp32, kind="ExternalInput")
    sd = []
    for s, K in ((0, d["K_LO"]), (1, d["K_HI"])):
        nch = NT * K
        sd.append(
            (
                nc.dram_tensor(f"g{s}", [128, nch * 8], i16, kind="ExternalInput"),
                nc.dram_tensor(f"r{s}", [128, nch], bf16, kind="ExternalInput"),
                nc.dram_tensor(f"as{s}", [128, nch, 1], fp32, kind="ExternalInput"),
                nc.dram_tensor(f"ad{s}", [128, nch, 1], fp32, kind="ExternalInput"),
            )
        )
    iota = nc.dram_tensor("iota", [128, CPC, 128], bf16, kind="ExternalInput")
    eye = nc.dram_tensor("eye", [128, 128], fp32, kind="ExternalInput")
    b1r = nc.dram_tensor("b1r", [128, C_OUT], fp32, kind="ExternalInput")
    out = nc.dram_tensor("out", [NP, C_OUT], fp32, kind="ExternalOutput")
    d = dict(d, iota_dram=iota)

    with tile.TileContext(nc) as tc:
        with (
            tc.tile_pool(name="oconst", bufs=1) as cpool,
            tc.tile_pool(name="ofin", bufs=3) as pool,
            tc.tile_pool(name="epsum", bufs=2, space="PSUM") as pp,
        ):
            b1_sb = cpool.tile([128, C_OUT], fp32)
            nc.sync.dma_start(b1_sb[:], b1r[:])

            def fin1(t, ps, dn):
                dnS = pool.tile([128, 1], fp32, tag="dnS", name="dnS")
                nc.vector.tensor_scalar_add(dnS[:], dn[:], EPS)
                recB = pool.tile([128, 1], fp32, tag="recB", name="recB")
                nc.vector.reciprocal(recB[:], dnS[:])
                O = pool.tile([128, C_OUT], fp32, tag="O", name="O")
                rb = recB[:].broadcast_to([128, C_OUT])
                nc.vector.tensor_tensor(O[:], ps[:, 0:C_OUT], rb, op=Alu.mult)
                nc.vector.tensor_tensor(O[:], O[:], b1_sb[:], op=Alu.add)
                nc.sync.dma_start(out[128 * t : 128 * (t + 1), :], O[:])

            _edge_pass(nc, tc, d, table1, sd, 64, fp32, C_OUT, 1, fin1, pp)
    nc.compile()
    return nc


# ------------------------------------------------------------ host plumbing


def _wrap_idx(idx):
    """idx[j] -> [j%16, j//16], replicated across the 8 q7 core groups."""
    a = idx.reshape(-1, 16).T.astype(np.int16)
    return np.tile(a, (8, 1))


def _prep_edges(edge_index, d):
    """Partition edges by dst shard; per core split by src < SPLIT (int16
    gather range), group by 128-row dst tile (sorted by dst), pad each
    (tile, stream) segment to the global max chunk count K_LO / K_HI.

    Returns per core, per stream: (wrapped_idx, rrT, src_global, dst_global)
    where src/dst_global are [NT*K, 128] int64 node ids (0 on pad slots)."""
    N, NLOC, NP = d["N"], d["NLOC"], d["NLOC_PAD"]
    SPLIT = d["SPLIT"]
    NT = NP // 128
    src = np.concatenate([edge_index[0], np.arange(N, dtype=np.int64)])
    dst = np.concatenate([edge_index[1], np.arange(N, dtype=np.int64)])
    core = dst // NLOC
    per_core = []
    kmax = [1, 1]
    for c in range(NCORES):
        m = core == c
        s, t = src[m], dst[m] - c * NLOC
        order = np.argsort(t, kind="stable")
        s, t = s[order], t[order]
        lo = s < SPLIT
        segs = []
        for sm, base in ((lo, 0), (~lo, SPLIT)):
            ss, tt = s[sm], t[sm]
            counts = np.bincount(tt // 128, minlength=NT)
            segs.append((ss, tt, counts, base))
        per_core.append(segs)
        for si in range(2):
            kmax[si] = max(kmax[si], int(np.ceil(per_core[c][si][2].max() / 128)))
    K_LO, K_HI = kmax
    res = []
    for c in range(NCORES):
        arrs = []
        for si, K in ((0, K_LO), (1, K_HI)):
            ss, tt, counts, base = per_core[c][si]
            g = np.zeros((NT, K * 128), np.int64)
            sg = np.zeros((NT, K * 128), np.int64)
            dg = np.zeros((NT, K * 128), np.int64)
            rr = np.full((NT, K * 128), -1.0, np.float32)
            offs = np.concatenate([[0], np.cumsum(counts)])
            for tl in range(NT):
                n = counts[tl]
                g[tl, :n] = ss[offs[tl] : offs[tl] + n] - base
                sg[tl, :n] = ss[offs[tl] : offs[tl] + n]
                dg[tl, :n] = tt[offs[tl] : offs[tl] + n] + c * NLOC
                rr[tl, :n] = (tt[offs[tl] : offs[tl] + n] - 128 * tl).astype(
                    np.float32
                )
            arrs.append(
                (
                    _wrap_idx(g.ravel()),
                    np.ascontiguousarray(rr.reshape(NT * K, 128).T),
                    sg.reshape(NT * K, 128),
                    dg.reshape(NT * K, 128),
                )
            )
        res.append(arrs)
    return K_LO, K_HI, res


def _build_A0(att_src, att_dst):
    H, DH = att_src.shape
    A = np.zeros((H * DH, 2 * H), np.float32)
    for h in range(H):
        A[h * DH : (h + 1) * DH, h] = att_src[h]
        A[h * DH : (h + 1) * DH, H + h] = att_dst[h]
    return A


def _bf16(a):
    import ml_dtypes

    return a.astype(ml_dtypes.bfloat16)


_cache = {}
LAST_PROFILE = {}


def _run(nc, in_maps, core_ids, label):
    trace = bool(int(os.environ.get("GAT_PROFILE", "0")))
    if trace:
        try:
            import sys

            import profile_hook

            profile_hook.install()
            import concourse.bass_utils as bu

            bu.upload_artifacts = lambda tmpdir: "local://skipped"
            br = run_bass_kernel_spmd(nc, in_maps, core_ids, trace=True)
            LAST_PROFILE[label] = br.exec_time_ns
            return br.results
        except Exception as e:  # fall back to untraced
            print(f"traced run failed ({e!r}); untraced retry", file=sys.stderr)
    br = run_bass_kernel_spmd(nc, in_maps, core_ids)
    LAST_PROFILE[label] = br.exec_time_ns
    return br.results


def kernel(x, edge_index, W0, att_src0, att_dst0, b0, W1, att_src1, att_dst1, b1):
    x = np.asarray(x, np.float32)
    edge_index = np.asarray(edge_index)
    d = _dims_full()
    d["N_TAB"] = d["N"]
    K_LO, K_HI, idx_arrs = _prep_edges(edge_index, d)
    d["K_LO"], d["K_HI"] = K_LO, K_HI

    key = (K_LO, K_HI)
    if key not in _cache:
        _cache[key] = (
            build_phase_a(d),
            build_layer0_edges(d),
            build_layer1_edges(d),
        )
    nc1, nc2, nc3 = _cache[key]

    N, NLOC, NP = d["N"], d["NLOC"], d["NLOC_PAD"]
    eye = np.eye(128, dtype=np.float32)
    iota = _bf16(
        np.tile(np.arange(128, dtype=np.float32)[None, None, :], (128, CPC, 1))
    )
    A0 = _build_A0(np.asarray(att_src0), np.asarray(att_dst0))
    W1f = np.asarray(W1, np.float32)
    W1b = _bf16(W1f)
    A1e = _bf16(
        W1f
        @ np.stack(
            [np.asarray(att_src1).ravel(), np.asarray(att_dst1).ravel()], axis=1
        ).astype(np.float32)
    )
    b0r = np.tile(np.asarray(b0, np.float32)[None, :], (128, 1))
    b1r = np.tile(np.asarray(b1, np.float32)[None, :], (128, 1))
    core_ids = list(range(NCORES))

    in1 = []
    for c in range(NCORES):
        xs = x[c * NLOC : (c + 1) * NLOC]
        xT = np.zeros((d["F_IN"], NP), np.float32)
        xT[:, :NLOC] = xs.T
        in1.append(dict(xT=xT, W0=np.asarray(W0, np.float32), A0=A0, eye=eye))
    r1 = _run(nc1, in1, core_ids, "l1")
    table0 = np.concatenate([r1[c]["table0"][:NLOC] for c in range(NCORES)], axis=0)
    atab0 = np.concatenate([r1[c]["atab0"][:NLOC] for c in range(NCORES)], axis=0)
    astab0, adtab0 = atab0[:, 0:4], atab0[:, 4:8]

    def edge_inputs(c, astab, adtab, extra):
        ins = dict(extra, iota=iota)
        for s in range(2):
            gw, rrT, sg, dg = idx_arrs[c][s]
            ins[f"g{s}"] = gw
            ins[f"r{s}"] = _bf16(rrT)
            ins[f"as{s}"] = np.ascontiguousarray(
                astab[sg].transpose(1, 0, 2)
            ).astype(np.float32)
            ins[f"ad{s}"] = np.ascontiguousarray(
                adtab[dg].transpose(1, 0, 2)
            ).astype(np.float32)
        return ins

    in2 = [
        edge_inputs(
            c,
            astab0,
            adtab0,
            dict(table0=table0, W1b=W1b, A1e=A1e, b0r=b0r, eye=eye),
        )
        for c in range(NCORES)
    ]
    r2 = _run(nc2, in2, core_ids, "l2")
    table1 = np.concatenate([r2[c]["table1"][:NLOC] for c in range(NCORES)], axis=0)
    atab1 = np.concatenate([r2[c]["atab1"][:NLOC] for c in range(NCORES)], axis=0)
    astab1, adtab1 = atab1[:, 0:1], atab1[:, 1:2]

    in3 = [
        edge_inputs(c, astab1, adtab1, dict(table1=table1, b1r=b1r, eye=eye))
        for c in range(NCORES)
    ]
    r3 = _run(nc3, in3, core_ids, "l3")
    out = np.concatenate([r3[c]["out"][:NLOC] for c in range(NCORES)], axis=0)
    return out


# revision 21
# speedup vs baseline: 1.6044x; 1.2529x over previous
"""Two-layer GAT (PyG-style GATConv x2) on 8 Trainium2 NeuronCores — v2.

Sharding: nodes (and incident edges, by destination) sharded across 8
cores; small weights replicated. Per-edge source rows fetched via SWDGE
dma_gather from a row-major node table in HBM (bf16 h rows for layer 0,
fp32 h1 rows for layer 1 — both exactly 512 B / 256 B per row, the
gather's minimum-efficient granularity). Edges are dst-sorted and
grouped per 128-row dst tile; each 128-edge chunk is segment-reduced
with a one-hot matmul into PSUM.

v2 changes vs v1 (2.14 ms):
 - Node tables carry ONLY features. Attention alphas are emitted as
   separate per-node fp32 tables; the host expands them per edge (pure
   fancy-indexing, same category as v1's alpha_dst expansion) so logits
   are assembled on-chip from two sequentially-streamed fp32 inputs.
   Layer-0 gather rows shrink 768 B -> 512 B.
 - Softmax denominators accumulate via a second tiny matmul per chunk
   (lhsT = exp-weights) instead of embedding weight columns in the
   gathered payload — kills v1's pathological strided DVE copy
   (~380 us/launch).
 - All PSUM->SBUF evacuation copies moved to the idle Scalar engine.
 - Gather calls: 16 chunks per call, striped round-robin across all 4
   SWDGE queues.
"""

import os

import numpy as np

import concourse.bacc as bacc
import concourse.mybir as mybir
from concourse import tile
from concourse.bass_utils import run_bass_kernel_spmd

fp32 = mybir.dt.float32
bf16 = mybir.dt.bfloat16
i16 = mybir.dt.int16
Alu = mybir.AluOpType
Act = mybir.ActivationFunctionType

NCORES = 8
NEG_SLOPE = 0.2
EPS = 1e-16
CPC = 8  # 128-edge chunks per gather call (1024 idx — SWDGE ring limit)


def _dims_full():
    return dict(
        N=50000,
        NLOC=6250,
        NLOC_PAD=6272,
        F_IN=256,
        HID=256,
        H=4,
        DH=64,
        C_OUT=64,
        ELEM0=256,  # bf16 h row -> 512 B
        ELEM1=64,  # fp32 h1 row -> 256 B
        SPLIT=32768,  # int16 gather-index split point
    )


# ---------------------------------------------------------------- launch 1


def build_phase_a(d):
    """Per core: h0 = x_shard @ W0 -> bf16 table0 rows; alphas -> fp32 atab0."""
    nc = bacc.Bacc(None, target_bir_lowering=False, debug=False, num_swdge_queues=4)
    NP, F, HID = d["NLOC_PAD"], d["F_IN"], d["HID"]
    assert F == 256 and HID == 256

    xT = nc.dram_tensor("xT", [F, NP], fp32, kind="ExternalInput")
    W0 = nc.dram_tensor("W0", [F, HID], fp32, kind="ExternalInput")
    A0 = nc.dram_tensor("A0", [HID, 8], fp32, kind="ExternalInput")
    eye = nc.dram_tensor("eye", [128, 128], fp32, kind="ExternalInput")
    table0 = nc.dram_tensor("table0", [NP, 256], bf16, kind="ExternalOutput")
    atab0 = nc.dram_tensor("atab0", [NP, 8], fp32, kind="ExternalOutput")

    TW = 512
    n_t = (NP + TW - 1) // TW

    with tile.TileContext(nc) as tc:
        with (
            tc.tile_pool(name="const", bufs=1) as cpool,
            tc.tile_pool(name="work", bufs=3) as pool,
            tc.tile_pool(name="psum", bufs=1, space="PSUM") as pp,
            tc.tile_pool(name="psum1", bufs=2, space="PSUM") as pp1,
        ):
            w0_sb = [
                cpool.tile([128, HID], fp32, tag=f"w0_{k}", name=f"w0_{k}")
                for k in range(2)
            ]
            a0_sb = [
                cpool.tile([128, 8], fp32, tag=f"a0_{k}", name=f"a0_{k}")
                for k in range(2)
            ]
            eye_sb = cpool.tile([128, 128], fp32)
            for k in range(2):
                nc.sync.dma_start(w0_sb[k][:], W0[128 * k : 128 * (k + 1), :])
                nc.sync.dma_start(a0_sb[k][:], A0[128 * k : 128 * (k + 1), :])
            nc.sync.dma_start(eye_sb[:], eye[:])

            for t in range(n_t):
                c0 = t * TW
                cw = min(TW, NP - c0)
                xt = [
                    pool.tile([128, TW], fp32, tag=f"xt{k}", name=f"xt{k}")
                    for k in range(2)
                ]
                for k in range(2):
                    nc.sync.dma_start(
                        xt[k][:, :cw], xT[128 * k : 128 * (k + 1), c0 : c0 + cw]
                    )
                hT = [
                    pool.tile([128, TW], fp32, tag=f"ht{m}", name=f"ht{m}")
                    for m in range(2)
                ]
                for m in range(2):
                    ps = pp.tile([128, TW], fp32, tag=f"ps{m}", name=f"ps{m}")
                    for k in range(2):
                        nc.tensor.matmul(
                            ps[:, :cw],
                            w0_sb[k][:, 128 * m : 128 * (m + 1)],
                            xt[k][:, :cw],
                            start=(k == 0),
                            stop=(k == 1),
                        )
                    nc.scalar.activation(hT[m][:, :cw], ps[:, :cw], Act.Copy)

                nq = (cw + 127) // 128
                for q in range(nq):
                    q0 = q * 128
                    qw = min(128, cw - q0)
                    pa = pp1.tile([128, 8], fp32, tag="pa")
                    for k in range(2):
                        nc.tensor.matmul(
                            pa[:qw, :],
                            hT[k][:, q0 : q0 + qw],
                            a0_sb[k][:],
                            start=(k == 0),
                            stop=(k == 1),
                        )
                    R = pool.tile([128, 256], bf16, tag="rows")
                    for m in range(2):
                        pt = pp1.tile([128, 128], fp32, tag=f"pt{m}", name=f"pt{m}")
                        nc.tensor.transpose(
                            pt[:qw, :], hT[m][:, q0 : q0 + qw], eye_sb[:]
                        )
                        nc.scalar.activation(
                            R[:qw, 128 * m : 128 * (m + 1)], pt[:qw, :], Act.Copy
                        )
                    paS = pool.tile([128, 8], fp32, tag="paS")
                    nc.scalar.activation(paS[:qw, :], pa[:qw, :], Act.Copy)
                    r0 = c0 + q0
                    nc.sync.dma_start(table0[r0 : r0 + qw, :], R[:qw, :])
                    nc.sync.dma_start(atab0[r0 : r0 + qw, :], paS[:qw, :])
    nc.compile()
    return nc


# ------------------------------------------------------------ edge machinery


def _edge_pass(nc, tc, d, table, streams_dram, elem, gdt, nfeat, nhead, fin, pp):
    """Dst-sorted edge pass. Per gather call (CPC chunks of 128 edges):
    fetch source rows (SWDGE gather, round-robin over the 4 queues),
    assemble logits from the two host-expanded per-edge alpha streams,
    leaky-relu + exp, build one-hot via is_equal, weight the payload.
    Per chunk: main one-hot matmul accumulates the weighted messages per
    dst tile; a second tiny matmul (lhsT = exp-weights) accumulates the
    softmax denominators as dnT [nhead, 128]."""
    NP, SPLIT, NROWS = d["NLOC_PAD"], d["SPLIT"], d["N_TAB"]
    KT_LO, KT_HI = d["KT_LO"], d["KT_HI"]
    NT = NP // 128
    qcnt = [0]

    with (
        tc.tile_pool(name="eidx", bufs=1) as ipool,
        tc.tile_pool(name="edge", bufs=3) as pool,
    ):
        iota_sb = ipool.tile([128, CPC, 128], bf16)
        nc.sync.dma_start(iota_sb[:], d["iota_dram"][:])
        streams = []
        for s, KT in ((0, KT_LO), (1, KT_HI)):
            gi_d, rr_d, as_d, ad_d = streams_dram[s]
            nch = sum(KT)
            gi = ipool.tile([128, nch * 8], i16, name=f"gi{s}")
            rr = ipool.tile([128, nch], bf16, name=f"rr{s}")
            asx = ipool.tile([128, nch, nhead], fp32, name=f"as{s}")
            adx = ipool.tile([128, nch, nhead], fp32, name=f"ad{s}")
            nc.sync.dma_start(gi[:], gi_d[:])
            nc.sync.dma_start(rr[:], rr_d[:])
            nc.sync.dma_start(asx[:], as_d[:])
            nc.sync.dma_start(adx[:], ad_d[:])
            base = table[0:SPLIT, :] if s == 0 else table[SPLIT:NROWS, :]
            off = [0]
            for k in KT:
                off.append(off[-1] + k)
            streams.append(
                dict(gi=gi, rr=rr, asx=asx, adx=adx, KT=KT, OFF=off, NCH=nch,
                     base=base, ncalls=0, tiles={})
            )

        def emit_call(st, call):
            c0 = call * CPC
            nch = min(CPC, st["NCH"] - c0)
            ne = nch * 128
            G = pool.tile([128, CPC, elem], gdt, tag="G", name="G", bufs=6)
            OH = pool.tile([128, CPC, 128], bf16, tag="OH", name="OH", bufs=6)
            nc.gpsimd.dma_gather(
                G[:, :nch, :],
                st["base"],
                st["gi"][:, c0 * 8 : c0 * 8 + ne // 16],
                ne,
                ne,
                elem,
                queue_num=qcnt[0] % 4,
            )
            qcnt[0] += 1
            ew = pool.tile([128, CPC, nhead], fp32, tag="ew", name="ew", bufs=6)
            nc.vector.tensor_tensor(
                ew[:, :nch, :],
                st["asx"][:, c0 : c0 + nch, :],
                st["adx"][:, c0 : c0 + nch, :],
                op=Alu.add,
            )
            nc.vector.scalar_tensor_tensor(
                ew[:, :nch, :],
                ew[:, :nch, :],
                NEG_SLOPE,
                ew[:, :nch, :],
                op0=Alu.mult,
                op1=Alu.max,
            )
            ewb = pool.tile([128, CPC, nhead], bf16, tag="ewb", name="ewb", bufs=6)
            nc.scalar.activation(ewb[:, :nch, :], ew[:, :nch, :], Act.Exp)
            rb = st["rr"][:, c0 : c0 + nch].unsqueeze(2).broadcast_to(
                [128, nch, 128]
            )
            nc.vector.tensor_tensor(
                OH[:, :nch, :], rb, iota_sb[:, :nch, :], op=Alu.is_equal
            )
            if gdt == bf16:
                gm = G[:, :nch, :].rearrange("p c (h e) -> p c h e", h=nhead)
                wb = (
                    ewb[:, :nch, :]
                    .unsqueeze(3)
                    .broadcast_to([128, nch, nhead, nfeat // nhead])
                )
                nc.vector.tensor_tensor(gm, gm, wb, op=Alu.mult)
                Gw = G
            else:
                Gw = pool.tile([128, CPC, elem], bf16, tag="Gw", name="Gw", bufs=6)
                wb = ewb[:, :nch, :].broadcast_to([128, nch, nfeat])
                nc.vector.tensor_tensor(Gw[:, :nch, :], G[:, :nch, :], wb,
                                        op=Alu.mult)
            return Gw, OH, ewb

        for t in range(NT):
            ps = pp.tile([128, nfeat], fp32, tag="ps", name="ps")
            dn = pp.tile([128, nhead], fp32, tag="dn", name="dn")
            first = True
            for st in streams:
                for k in range(st["KT"][t]):
                    c = st["OFF"][t] + k
                    call, cin = c // CPC, c % CPC
                    if call >= st["ncalls"]:
                        st["tiles"][call] = emit_call(st, call)
                        st["ncalls"] = call + 1
                        st["tiles"].pop(call - 5, None)
                    Gw, OH, ewb = st["tiles"][call]
                    last = st is streams[1] and k == st["KT"][t] - 1
                    nc.tensor.matmul(
                        ps[:],
                        OH[:, cin, :],
                        Gw[:, cin, 0:nfeat],
                        start=first,
                        stop=last,
                        skip_group_check=True,
                    )
                    nc.tensor.matmul(
                        dn[:],
                        OH[:, cin, :],
                        ewb[:, cin, :],
                        start=first,
                        stop=last,
                        skip_group_check=True,
                    )
                    first = False
            fin(t, ps, dn)


# ---------------------------------------------------------------- launch 2


def build_layer0_edges(d):
    """Layer-0 edge pass with fused finalize (softmax-div + bias + ELU),
    then h1 = h0' @ W1 -> fp32 table1 rows + fp32 atab1."""
    nc = bacc.Bacc(None, target_bir_lowering=False, debug=False, num_swdge_queues=4)
    NP = d["NLOC_PAD"]
    HID, C_OUT, H, DH = d["HID"], d["C_OUT"], d["H"], d["DH"]
    NT = NP // 128

    table0 = nc.dram_tensor("table0", [d["N_TAB"], 256], bf16, kind="ExternalInput")
    sd = []
    for s, KT in ((0, d["KT_LO"]), (1, d["KT_HI"])):
        nch = sum(KT)
        sd.append(
            (
                nc.dram_tensor(f"g{s}", [128, nch * 8], i16, kind="ExternalInput"),
                nc.dram_tensor(f"r{s}", [128, nch], bf16, kind="ExternalInput"),
                nc.dram_tensor(f"as{s}", [128, nch, H], fp32, kind="ExternalInput"),
                nc.dram_tensor(f"ad{s}", [128, nch, H], fp32, kind="ExternalInput"),
            )
        )
    iota = nc.dram_tensor("iota", [128, CPC, 128], bf16, kind="ExternalInput")
    W1b = nc.dram_tensor("W1b", [HID, C_OUT], bf16, kind="ExternalInput")
    A1e = nc.dram_tensor("A1e", [HID, 2], bf16, kind="ExternalInput")
    b0r = nc.dram_tensor("b0r", [128, HID], fp32, kind="ExternalInput")
    eye = nc.dram_tensor("eye", [128, 128], fp32, kind="ExternalInput")
    table1 = nc.dram_tensor("table1", [NP, 64], fp32, kind="ExternalOutput")
    atab1 = nc.dram_tensor("atab1", [NP, 2], fp32, kind="ExternalOutput")
    d = dict(d, iota_dram=iota)

    with tile.TileContext(nc) as tc:
        with (
            tc.tile_pool(name="fconst", bufs=1) as cpool,
            tc.tile_pool(name="fin", bufs=3) as pool,
            tc.tile_pool(name="h0all", bufs=1) as hpool,
            tc.tile_pool(name="epsum", bufs=2, space="PSUM") as pp,
        ):
            b0_sb = cpool.tile([128, HID], fp32)
            nc.sync.dma_start(b0_sb[:], b0r[:])
            eye_sb = cpool.tile([128, 128], fp32)
            nc.sync.dma_start(eye_sb[:], eye[:])
            H0 = hpool.tile([128, NT, HID], fp32)

            def fin0(t, ps, dn):
                dnS = pool.tile([128, H], fp32, tag="dnS", name="dnS")
                nc.vector.tensor_scalar_add(dnS[:], dn[:], EPS)
                recB = pool.tile([128, H], fp32, tag="recB", name="recB")
                nc.vector.reciprocal(recB[:], dnS[:])
                f4 = ps[:, 0:HID].rearrange("p (h e) -> p h e", h=H)
                rb = recB[:].unsqueeze(2).broadcast_to([128, H, DH])
                hrow = H0[:, t, :]
                nc.vector.tensor_tensor(
                    hrow.rearrange("p (h e) -> p h e", h=H), f4, rb, op=Alu.mult
                )
                nc.vector.tensor_tensor(hrow, hrow, b0_sb[:], op=Alu.add)
                # ELU on the Scalar engine: exp(min(x,0)) = Exp(-Relu(-x)),
                # max(x,0) = Relu(x); only the final combine stays on DVE.
                tn = pool.tile([128, HID], fp32, tag="tn", name="tn")
                nc.scalar.activation(tn[:], hrow, Act.Relu, scale=-1.0)
                nc.scalar.activation(tn[:], tn[:], Act.Exp, scale=-1.0)
                tp = pool.tile([128, HID], fp32, tag="tp", name="tp")
                nc.scalar.activation(tp[:], hrow, Act.Relu)
                nc.vector.scalar_tensor_tensor(
                    hrow, tn[:], -1.0, tp[:], op0=Alu.add, op1=Alu.add
                )

            _edge_pass(nc, tc, d, table0, sd, 256, bf16, HID, H, fin0, pp)

            with (
                tc.tile_pool(name="tb1", bufs=3) as tpool,
                tc.tile_pool(name="tb1psum", bufs=1, space="PSUM") as pp2,
            ):
                w1_sb = [
                    cpool.tile([128, C_OUT], bf16, tag=f"w1_{k}", name=f"w1_{k}")
                    for k in range(2)
                ]
                a1e_sb = [
                    cpool.tile([128, 2], bf16, tag=f"a1e_{k}", name=f"a1e_{k}")
                    for k in range(2)
                ]
                for k in range(2):
                    nc.sync.dma_start(w1_sb[k][:], W1b[128 * k : 128 * (k + 1), :])
                    nc.sync.dma_start(a1e_sb[k][:], A1e[128 * k : 128 * (k + 1), :])

                for r in range(NT):
                    h0T = [
                        tpool.tile([128, 128], bf16, tag=f"h0T{k}", name=f"h0T{k}")
                        for k in range(2)
                    ]
                    for k in range(2):
                        pt = pp2.tile([128, 128], fp32, tag="pt", name="pt")
                        nc.tensor.transpose(
                            pt[:], H0[:, r, 128 * k : 128 * (k + 1)], eye_sb[:]
                        )
                        nc.scalar.activation(h0T[k][:], pt[:], Act.Copy)
                    pr1 = pp2.tile([128, C_OUT], fp32, tag="pr1", name="pr1")
                    pr2 = pp2.tile([128, 2], fp32, tag="pr2", name="pr2")
                    for k in range(2):
                        nc.tensor.matmul(
                            pr1[:],
                            h0T[k][:],
                            w1_sb[k][:],
                            start=(k == 0),
                            stop=(k == 1),
                            skip_group_check=True,
                        )
                        nc.tensor.matmul(
                            pr2[:],
                            h0T[k][:],
                            a1e_sb[k][:],
                            start=(k == 0),
                            stop=(k == 1),
                            skip_group_check=True,
                        )
                    R1 = tpool.tile([128, C_OUT], fp32, tag="R1", name="R1")
                    nc.scalar.activation(R1[:], pr1[:], Act.Copy)
                    palS = tpool.tile([128, 2], fp32, tag="palS", name="palS")
                    nc.scalar.activation(palS[:], pr2[:], Act.Copy)
                    nc.sync.dma_start(table1[128 * r : 128 * (r + 1), :], R1[:])
                    nc.sync.dma_start(atab1[128 * r : 128 * (r + 1), :], palS[:])
    nc.compile()
    return nc


# ---------------------------------------------------------------- launch 3


def build_layer1_edges(d):
    """Layer-1 edge pass with fused finalize -> output shard."""
    nc = bacc.Bacc(None, target_bir_lowering=False, debug=False, num_swdge_queues=4)
    NP, C_OUT = d["NLOC_PAD"], d["C_OUT"]
    NT = NP // 128

    table1 = nc.dram_tensor("table1", [d["N_TAB"], 64], fp32, kind="ExternalInput")
    sd = []
    for s, KT in ((0, d["KT_LO"]), (1, d["KT_HI"])):
        nch = sum(KT)
        sd.append(
            (
                nc.dram_tensor(f"g{s}", [128, nch * 8], i16, kind="ExternalInput"),
                nc.dram_tensor(f"r{s}", [128, nch], bf16, kind="ExternalInput"),
                nc.dram_tensor(f"as{s}", [128, nch, 1], fp32, kind="ExternalInput"),
                nc.dram_tensor(f"ad{s}", [128, nch, 1], fp32, kind="ExternalInput"),
            )
        )
    iota = nc.dram_tensor("iota", [128, CPC, 128], bf16, kind="ExternalInput")
    eye = nc.dram_tensor("eye", [128, 128], fp32, kind="ExternalInput")
    b1r = nc.dram_tensor("b1r", [128, C_OUT], fp32, kind="ExternalInput")
    out = nc.dram_tensor("out", [NP, C_OUT], fp32, kind="ExternalOutput")
    d = dict(d, iota_dram=iota)

    with tile.TileContext(nc) as tc:
        with (
            tc.tile_pool(name="oconst", bufs=1) as cpool,
            tc.tile_pool(name="ofin", bufs=3) as pool,
            tc.tile_pool(name="epsum", bufs=2, space="PSUM") as pp,
        ):
            b1_sb = cpool.tile([128, C_OUT], fp32)
            nc.sync.dma_start(b1_sb[:], b1r[:])

            def fin1(t, ps, dn):
                dnS = pool.tile([128, 1], fp32, tag="dnS", name="dnS")
                nc.vector.tensor_scalar_add(dnS[:], dn[:], EPS)
                recB = pool.tile([128, 1], fp32, tag="recB", name="recB")
                nc.vector.reciprocal(recB[:], dnS[:])
                O = pool.tile([128, C_OUT], fp32, tag="O", name="O")
                rb = recB[:].broadcast_to([128, C_OUT])
                nc.vector.tensor_tensor(O[:], ps[:, 0:C_OUT], rb, op=Alu.mult)
                nc.vector.tensor_tensor(O[:], O[:], b1_sb[:], op=Alu.add)
                nc.sync.dma_start(out[128 * t : 128 * (t + 1), :], O[:])

            _edge_pass(nc, tc, d, table1, sd, 64, fp32, C_OUT, 1, fin1, pp)
    nc.compile()
    return nc


# ------------------------------------------------------------ host plumbing


def _wrap_idx(idx):
    """idx[j] -> [j%16, j//16], replicated across the 8 q7 core groups."""
    a = idx.reshape(-1, 16).T.astype(np.int16)
    return np.tile(a, (8, 1))


def _prep_edges(edge_index, d):
    """Partition edges by dst shard; per core split by src < SPLIT (int16
    gather range), group by 128-row dst tile (sorted by dst), pad each
    (tile, stream) segment to the global max chunk count K_LO / K_HI.

    Per-tile chunk counts KT[s][t] = max over cores (not a global max) —
    cuts ~10%% padding. Returns per core, per stream:
    (wrapped_idx, rrT, src_global, dst_global), slot arrays [nch, 128]."""
    N, NLOC, NP = d["N"], d["NLOC"], d["NLOC_PAD"]
    SPLIT = d["SPLIT"]
    NT = NP // 128
    src = np.concatenate([edge_index[0], np.arange(N, dtype=np.int64)])
    dst = np.concatenate([edge_index[1], np.arange(N, dtype=np.int64)])
    core = dst // NLOC
    per_core = []
    counts_all = np.zeros((NCORES, 2, NT), np.int64)
    for c in range(NCORES):
        m = core == c
        s, t = src[m], dst[m] - c * NLOC
        order = np.argsort(t, kind="stable")
        s, t = s[order], t[order]
        lo = s < SPLIT
        segs = []
        for si, (sm, base) in enumerate(((lo, 0), (~lo, SPLIT))):
            ss, tt = s[sm], t[sm]
            counts = np.bincount(tt // 128, minlength=NT)
            segs.append((ss, tt, counts, base))
            counts_all[c, si] = counts
        per_core.append(segs)
    KT = [
        tuple(
            int(v)
            for v in np.maximum(
                1, -(-counts_all[:, si].max(axis=0) // 128)
            )
        )
        for si in range(2)
    ]
    OFF = [np.concatenate([[0], np.cumsum(K)]).astype(int) for K in KT]
    res = []
    for c in range(NCORES):
        arrs = []
        for si in range(2):
            ss, tt, counts, base = per_core[c][si]
            nch = int(OFF[si][NT])
            g = np.zeros((nch * 128,), np.int64)
            sg = np.zeros((nch * 128,), np.int64)
            dg = np.zeros((nch * 128,), np.int64)
            rr = np.full((nch * 128,), -1.0, np.float32)
            offs = np.concatenate([[0], np.cumsum(counts)])
            for tl in range(NT):
                n = counts[tl]
                s0 = int(OFF[si][tl]) * 128
                g[s0 : s0 + n] = ss[offs[tl] : offs[tl] + n] - base
                sg[s0 : s0 + n] = ss[offs[tl] : offs[tl] + n]
                dg[s0 : s0 + n] = tt[offs[tl] : offs[tl] + n] + c * NLOC
                rr[s0 : s0 + n] = (tt[offs[tl] : offs[tl] + n] - 128 * tl).astype(
                    np.float32
                )
            arrs.append(
                (
                    _wrap_idx(g),
                    np.ascontiguousarray(rr.reshape(nch, 128).T),
                    sg.reshape(nch, 128),
                    dg.reshape(nch, 128),
                )
            )
        res.append(arrs)
    return KT[0], KT[1], res


def _build_A0(att_src, att_dst):
    H, DH = att_src.shape
    A = np.zeros((H * DH, 2 * H), np.float32)
    for h in range(H):
        A[h * DH : (h + 1) * DH, h] = att_src[h]
        A[h * DH : (h + 1) * DH, H + h] = att_dst[h]
    return A


def _bf16(a):
    import ml_dtypes

    return a.astype(ml_dtypes.bfloat16)


_cache = {}
LAST_PROFILE = {}


def _run(nc, in_maps, core_ids, label):
    trace = bool(int(os.environ.get("GAT_PROFILE", "0")))
    if trace:
        try:
            import sys

            import profile_hook

            profile_hook.install()
            import concourse.bass_utils as bu

            bu.upload_artifacts = lambda tmpdir: "local://skipped"
            br = run_bass_kernel_spmd(nc, in_maps, core_ids, trace=True)
            LAST_PROFILE[label] = br.exec_time_ns
            return br.results
        except Exception as e:  # fall back to untraced
            print(f"traced run failed ({e!r}); untraced retry", file=sys.stderr)
    br = run_bass_kernel_spmd(nc, in_maps, core_ids)
    LAST_PROFILE[label] = br.exec_time_ns
    return br.results


def kernel(x, edge_index, W0, att_src0, att_dst0, b0, W1, att_src1, att_dst1, b1):
    x = np.asarray(x, np.float32)
    edge_index = np.asarray(edge_index)
    d = _dims_full()
    d["N_TAB"] = d["N"]
    KT_LO, KT_HI, idx_arrs = _prep_edges(edge_index, d)
    d["KT_LO"], d["KT_HI"] = KT_LO, KT_HI

    key = (KT_LO, KT_HI)
    if key not in _cache:
        _cache[key] = (
            build_phase_a(d),
            build_layer0_edges(d),
            build_layer1_edges(d),
        )
    nc1, nc2, nc3 = _cache[key]

    N, NLOC, NP = d["N"], d["NLOC"], d["NLOC_PAD"]
    eye = np.eye(128, dtype=np.float32)
    iota = _bf16(
        np.tile(np.arange(128, dtype=np.float32)[None, None, :], (128, CPC, 1))
    )
    A0 = _build_A0(np.asarray(att_src0), np.asarray(att_dst0))
    W1f = np.asarray(W1, np.float32)
    W1b = _bf16(W1f)
    A1e = _bf16(
        W1f
        @ np.stack(
            [np.asarray(att_src1).ravel(), np.asarray(att_dst1).ravel()], axis=1
        ).astype(np.float32)
    )
    b0r = np.tile(np.asarray(b0, np.float32)[None, :], (128, 1))
    b1r = np.tile(np.asarray(b1, np.float32)[None, :], (128, 1))
    core_ids = list(range(NCORES))

    in1 = []
    for c in range(NCORES):
        xs = x[c * NLOC : (c + 1) * NLOC]
        xT = np.zeros((d["F_IN"], NP), np.float32)
        xT[:, :NLOC] = xs.T
        in1.append(dict(xT=xT, W0=np.asarray(W0, np.float32), A0=A0, eye=eye))
    r1 = _run(nc1, in1, core_ids, "l1")
    table0 = np.concatenate([r1[c]["table0"][:NLOC] for c in range(NCORES)], axis=0)
    atab0 = np.concatenate([r1[c]["atab0"][:NLOC] for c in range(NCORES)], axis=0)
    astab0, adtab0 = atab0[:, 0:4], atab0[:, 4:8]

    def edge_inputs(c, astab, adtab, extra):
        ins = dict(extra, iota=iota)
        for s in range(2):
            gw, rrT, sg, dg = idx_arrs[c][s]
            ins[f"g{s}"] = gw
            ins[f"r{s}"] = _bf16(rrT)
            ins[f"as{s}"] = np.ascontiguousarray(
                astab[sg].transpose(1, 0, 2)
            ).astype(np.float32)
            ins[f"ad{s}"] = np.ascontiguousarray(
                adtab[dg].transpose(1, 0, 2)
            ).astype(np.float32)
        return ins

    in2 = [
        edge_inputs(
            c,
            astab0,
            adtab0,
            dict(table0=table0, W1b=W1b, A1e=A1e, b0r=b0r, eye=eye),
        )
        for c in range(NCORES)
    ]
    r2 = _run(nc2, in2, core_ids, "l2")
    table1 = np.concatenate([r2[c]["table1"][:NLOC] for c in range(NCORES)], axis=0)
    atab1 = np.concatenate([r2[c]["atab1"][:NLOC] for c in range(NCORES)], axis=0)
    astab1, adtab1 = atab1[:, 0:1], atab1[:, 1:2]

    in3 = [
        edge_inputs(c, astab1, adtab1, dict(table1=table1, b1r=b1r, eye=eye))
        for c in range(NCORES)
    ]
    r3 = _run(nc3, in3, core_ids, "l3")
    out = np.concatenate([r3[c]["out"][:NLOC] for c in range(NCORES)], axis=0)
    return out
